# revision 1
# baseline (speedup 1.0000x reference)
"""Trainium2 Bass kernel for CausalSelfAttention (RoPE + GQA), 8-core SPMD.

Sharding: 8 cores = 4 batches x 2 query-halves. Each core owns four
query-256-blocks paired {i, 7-i} so causal work is balanced. Keys are
PERMUTED per core: block order = [own q-blocks (desc causal depth), then
remaining blocks ascending]. With that order, slot s consumes exactly the
static key-chunk range [2s, 2s+PAD_s), its diagonal chunks are 2s..2s+1
(emitted last so one bf16 mask multiply per slot covers diag+pad), and the
first 1024 key columns ARE the core's queries -- so Q-projection re-reads
the same xkT input and the K RoPE tables double as Q tables. Every core
runs an identical instruction stream; all variation is input data.

Device pipeline per core:
  QKV projections in fp32r (inputs pre-rounded on host, DMA'd straight into
  f32r tiles -- no cast ops); RoPE via double projection (normal +
  pair-swapped weights on PE) + two table multiplies (DVE) + adds (GPSIMD).
  Attention per (head, slot): S^T = K^T.T @ Q^T in fp32r with keys on
  partitions, exp on ScalarE (PSUM -> bf16, scale=1/8), one bf16 mask
  multiply, P.V as bf16 matmul with a ones-augmented V column yielding the
  softmax denominator free, reciprocal + gpsimd partition-broadcast divide.
  fp32r output projection interleaved with the last attention slots.
  PSUM banks are partitioned so attention overlaps the projection phases.
"""
import sys

sys.path.insert(0, "/opt/trn_rl_repo")

import numpy as np
import ml_dtypes

B, T, C = 4, 2048, 576
H, HKV, D = 9, 3, 64
THETA = 10000.0
QB = 256                      # query block
TQ = 1024                     # queries per core
SLOT_PAD = [16, 12, 8, 4]     # padded key-chunk counts per slot
QBLOCKS = [[7, 5, 2, 0], [6, 4, 3, 1]]   # q-256-block ids per half j
KEYORDER = [[7, 5, 2, 0, 1, 3, 4, 6], [6, 4, 3, 1, 0, 2, 5, 7]]
CCX = [(0, 128), (128, 128), (256, 128), (384, 128), (512, 65)]   # x chunks (577 rows incl ones)
CCQ = [(0, 128), (128, 128), (256, 128), (384, 128), (512, 64)]   # 576-row chunks
MM = [(0, 128), (128, 128), (256, 128), (384, 128), (512, 64)]    # output-dim chunks of 576


def _slot_seq(s):
    """Key-chunk emission order for slot s: fulls, then the two diag chunks."""
    return list(range(2 * s + 2, 2 * s + SLOT_PAD[s])) + [2 * s, 2 * s + 1]


_PROG = None


def _rne12(x):
    """Round fp32 to f32r (RNE, drop 12 mantissa bits) -- matches TRN2."""
    b = np.ascontiguousarray(x, np.float32).view(np.uint32).astype(np.uint64)
    lsb = (b >> np.uint64(12)) & np.uint64(1)
    r = (b + np.uint64(2047) + lsb) >> np.uint64(12) << np.uint64(12)
    return (r & np.uint64(0xFFFFFFFF)).astype(np.uint32).view(np.float32)


def _build_program(ablate=(), reps=1):
    import concourse.bacc as bacc
    import concourse.mybir as mybir
    import concourse.tile as tile

    dt = mybir.dt
    f32, f32r, bf16 = dt.float32, dt.float32r, dt.bfloat16
    AF = mybir.ActivationFunctionType

    nc = bacc.Bacc("TRN2", target_bir_lowering=False, debug=False, num_devices=8)

    def inp(name, shape, d=f32):
        return nc.declare_dram_parameter(name, shape, d, isOutput=False)

    xkT = inp("xkT", [577, T], f32r)
    wqT = inp("wqT", [C, C], f32r)
    wqsT = inp("wqsT", [C, C], f32r)
    wkT = inp("wkT", [C, HKV * D], f32r)
    wksT = inp("wksT", [C, HKV * D], f32r)
    wvT = inp("wvT", [577, 260], f32r)
    woT = inp("woT", [C, C], f32r)
    c2k = inp("c2k", [128, T])
    s2k = inp("s2k", [128, T])
    masksp = inp("masks", [16 * 128, QB], bf16)
    yT = nc.declare_dram_parameter("yT", [C, TQ], f32, isOutput=True)

    with tile.TileContext(nc) as tc:
      for _rep in range(reps):
            with (
                tc.tile_pool(name="const", bufs=1) as cp,
                tc.tile_pool(name="tab", bufs=1) as tab,
                tc.tile_pool(name="rope", bufs=2) as rp,
                tc.tile_pool(name="pwork", bufs=3) as pw,
                # attention PSUM lives on banks disjoint from the phase pools so
                # attention can overlap the projections
                tc.tile_pool(name="psS", bufs=2, space="PSUM") as psS,
                tc.tile_pool(name="psY", bufs=2, space="PSUM") as psY,
            ):
                def load_w(pool, param, chunks, cols, tag):
                    tiles = []
                    for i, (k0, kl) in enumerate(chunks):
                        t = pool.tile([128, cols], f32r, tag=f"{tag}{i}", name=f"{tag}{i}")
                        nc.sync.dma_start(t[:kl, :], param[k0:k0 + kl, :])
                        tiles.append(t)
                    return tiles

                wo_r = load_w(cp, woT, MM, C, "wo")
                m_b = cp.tile([128, 16 * QB], bf16, tag="masks", name="masks")
                for i in range(16):
                    nc.sync.dma_start(m_b[:, i * QB:(i + 1) * QB],
                                      masksp[i * 128:(i + 1) * 128, :])
                c2k_t = tab.tile([128, T], f32, tag="c2k", name="c2k")
                s2k_t = tab.tile([128, T], f32, tag="s2k", name="s2k")
                nc.sync.dma_start(c2k_t[:], c2k[:])
                nc.sync.dma_start(s2k_t[:], s2k[:])

                # persistent projection outputs
                kt_h = [cp.tile([64, T], f32r, tag=f"kt{g}", name=f"kt{g}")
                        for g in range(HKV)]
                qth = [cp.tile([64, TQ], f32r, tag=f"qth{h}", name=f"qth{h}")
                       for h in range(H)]
                v_t = [cp.tile([128, 260], f32r, tag=f"v{c}", name=f"v{c}")
                       for c in range(16)]
                ypr = [cp.tile([128, TQ], f32r, tag=f"ypr{p}", name=f"ypr{p}")
                       for p in range(5)]

                def rope(ps, pssw, rows, cols0, n, dsts):
                    """dsts[bi][:, cols0:+n] = ps*c2 + pssw*s2, per 64-row block."""
                    t1 = rp.tile([128, 512], f32r, tag="rope1", name="rope1")
                    t2 = rp.tile([128, 512], f32r, tag="rope2", name="rope2")
                    nc.vector.tensor_mul(t1[:rows, :n], ps[:rows, :n],
                                         c2k_t[:rows, cols0:cols0 + n])
                    nc.vector.tensor_mul(t2[:rows, :n], pssw[:rows, :n],
                                         s2k_t[:rows, cols0:cols0 + n])
                    for bi, dt_ in enumerate(dsts):
                        nc.gpsimd.tensor_add(dt_[0:64, cols0:cols0 + n],
                                             t1[64 * bi:64 * bi + 64, :n],
                                             t2[64 * bi:64 * bi + 64, :n])

                # ---------- phase 1: K-proj+rope, V-proj (keys, 4 windows) -----
                if "phase1" not in ablate:
                  with (
                      tc.tile_pool(name="wkv", bufs=1) as wkvp,
                      tc.tile_pool(name="psA", bufs=1, space="PSUM") as psA,
                      tc.tile_pool(name="psB", bufs=1, space="PSUM") as psB,
                  ):
                    wk_r = load_w(wkvp, wkT, CCQ, HKV * D, "wk")
                    wks_r = load_w(wkvp, wksT, CCQ, HKV * D, "wks")
                    wv_r = load_w(wkvp, wvT, CCX, 260, "wv")
                    with tc.tile_pool(name="xk", bufs=2) as xkp:
                        for nn_ in range(4):
                            xk_r = []
                            for i, (k0, kl) in enumerate(CCX):
                                t = xkp.tile([128, 512], f32r, tag=f"xk{i}",
                                             name=f"xk{i}")
                                nc.sync.dma_start(
                                    t[:kl, :],
                                    xkT[k0:k0 + kl, 512 * nn_:512 * (nn_ + 1)])
                                xk_r.append(t)

                            for mi, (mc0, mrows) in enumerate([(0, 128), (128, 64)]):
                                ps = psA.tile([128, 512], f32, tag="pja", name="pja")
                                pss = psB.tile([128, 512], f32, tag="pjb", name="pjb")
                                for ci, (k0, kl) in enumerate(CCQ):
                                    nc.tensor.matmul(
                                        ps[:mrows, :],
                                        wk_r[ci][:kl, mc0:mc0 + mrows],
                                        xk_r[ci][:kl, :],
                                        start=(ci == 0), stop=(ci == 4))
                                for ci, (k0, kl) in enumerate(CCQ):
                                    nc.tensor.matmul(
                                        pss[:mrows, :],
                                        wks_r[ci][:kl, mc0:mc0 + mrows],
                                        xk_r[ci][:kl, :],
                                        start=(ci == 0), stop=(ci == 4))
                                rope(ps, pss, mrows, 512 * nn_, 512,
                                     [kt_h[0], kt_h[1]] if mi == 0 else [kt_h[2]])

                            for ti in range(4):
                                t_ = 4 * nn_ + ti
                                vpool = psA if ti % 2 == 0 else psB
                                vtag = "pja" if ti % 2 == 0 else "pjb"
                                ps = vpool.tile([128, 512], f32, tag=vtag, name=vtag)
                                for ci, (k0, kl) in enumerate(CCX):
                                    nc.tensor.matmul(
                                        ps[:, :260],
                                        xk_r[ci][:kl, 128 * ti:128 * (ti + 1)],
                                        wv_r[ci][:kl, :],
                                        start=(ci == 0), stop=(ci == 4))
                                nc.scalar.activation(v_t[t_][:], ps[:, :260], AF.Copy)

                # ---------- attention helpers ------------------------------
                def attn_slot(s):
                    seq = _slot_seq(s)
                    n = len(seq)
                    for h in range(H):
                        g = h // 3
                        hp, hr = h // 2, 64 * (h % 2)
                        y_ps = psY.tile([65, QB], f32, tag="ypsum", name="ypsum")
                        for sc in range(n // 4):
                            sp = psS.tile([128, 4 * QB], f32, tag="scores",
                                          name="scores")
                            for i in range(4):
                                c = seq[4 * sc + i]
                                nc.tensor.matmul(
                                    sp[:, QB * i:QB * (i + 1)],
                                    kt_h[g][0:64, 128 * c:128 * (c + 1)],
                                    qth[h][0:64, QB * s:QB * (s + 1)],
                                    start=True, stop=True)
                            p_b = pw.tile([128, 4 * QB], f32r, tag="p", name="p")
                            nc.scalar.activation(p_b[:], sp[:], AF.Exp, scale=0.125)
                            if sc == n // 4 - 1:
                                nc.vector.tensor_mul(
                                    p_b[:], p_b[:],
                                    m_b[:, 1024 * s:1024 * (s + 1)])
                            for i in range(4):
                                c = seq[4 * sc + i]
                                nc.tensor.matmul(
                                    y_ps[:], v_t[c][:, 65 * g:65 * g + 65],
                                    p_b[:, QB * i:QB * (i + 1)],
                                    start=(4 * sc + i == 0),
                                    stop=(4 * sc + i == n - 1))
                        recip = pw.tile([1, QB], f32, tag="recip", name="recip")
                        nc.vector.reciprocal(recip[:], y_ps[64:65, :])
                        rb_sb = pw.tile([D, QB], f32, tag="rb", name="rb")
                        nc.gpsimd.partition_broadcast(rb_sb[:], recip[:], D)
                        nc.vector.tensor_mul(
                            ypr[hp][hr:hr + 64, QB * s:QB * (s + 1)],
                            y_ps[0:64, :], rb_sb[:])

                # ---------- phase 2 (Q-proj) interleaved with attention --------
                if "phase2" not in ablate:
                  with (
                      tc.tile_pool(name="wq", bufs=1) as wqp,
                      tc.tile_pool(name="psA2", bufs=1, space="PSUM") as psA,
                      tc.tile_pool(name="psB2", bufs=1, space="PSUM") as psB,
                  ):
                    wq_r = load_w(wqp, wqT, CCQ, C, "wq")
                    wqs_r = load_w(wqp, wqsT, CCQ, C, "wqs")
                    with tc.tile_pool(name="xq", bufs=2) as xqp:
                        for nn_ in (1, 0):      # window 1 feeds slots 3,2 (run first)
                            xq_r = []
                            for i, (k0, kl) in enumerate(CCQ):
                                t = xqp.tile([128, 512], f32r, tag=f"xq{i}",
                                             name=f"xq{i}")
                                nc.sync.dma_start(
                                    t[:kl, :],
                                    xkT[k0:k0 + kl, 512 * nn_:512 * (nn_ + 1)])
                                xq_r.append(t)

                            for m, (mc0, mrows) in enumerate(MM):
                                ps = psA.tile([128, 512], f32, tag="pja", name="pja")
                                pss = psB.tile([128, 512], f32, tag="pjb", name="pjb")
                                for ci, (k0, kl) in enumerate(CCQ):
                                    nc.tensor.matmul(
                                        ps[:mrows, :],
                                        wq_r[ci][:kl, mc0:mc0 + mrows],
                                        xq_r[ci][:kl, :],
                                        start=(ci == 0), stop=(ci == 4))
                                for ci, (k0, kl) in enumerate(CCQ):
                                    nc.tensor.matmul(
                                        pss[:mrows, :],
                                        wqs_r[ci][:kl, mc0:mc0 + mrows],
                                        xq_r[ci][:kl, :],
                                        start=(ci == 0), stop=(ci == 4))
                                dsts = ([qth[2 * m], qth[2 * m + 1]] if m < 4
                                        else [qth[8]])
                                rope(ps, pss, mrows, 512 * nn_, 512, dsts)

                            if nn_ == 1 and "attn" not in ablate:
                                # slots 3,2 only need Q window 1 -- emit them now
                                # so their PE/ACT work overlaps Q window 0
                                attn_slot(3)
                                attn_slot(2)

                # ---------- remaining attention + out-proj ---------------------
                with tc.tile_pool(name="psR", bufs=2, space="PSUM") as psR:
                    def oproj(nn_):
                        for m, (mc0, mrows) in enumerate(MM):
                            ps = psR.tile([128, 512], f32, tag="pjr", name="pjr")
                            for p, (pc0, pl) in enumerate(MM):
                                nc.tensor.matmul(
                                    ps[:mrows, :],
                                    wo_r[p][:pl, mc0:mc0 + mrows],
                                    ypr[p][:pl, 512 * nn_:512 * (nn_ + 1)],
                                    start=(p == 0), stop=(p == 4))
                            ost = pw.tile([128, 512], f32, tag="ostage", name="ostage")
                            nc.vector.tensor_copy(ost[:mrows, :], ps[:mrows, :])
                            nc.sync.dma_start(
                                yT[mc0:mc0 + mrows, 512 * nn_:512 * (nn_ + 1)],
                                ost[:mrows, :])

                    if "oproj" not in ablate:
                        oproj(1)
                    if "attn" not in ablate:
                        attn_slot(1)
                        attn_slot(0)
                    if "oproj" not in ablate:
                        oproj(0)

    nc.compile()
    return nc


def _get_program():
    global _PROG
    if _PROG is None:
        _PROG = _build_program()
    return _PROG


def _neox_perm(nheads, swap=False):
    p = []
    for h in range(nheads):
        ev = [64 * h + 2 * j for j in range(32)]
        od = [64 * h + 2 * j + 1 for j in range(32)]
        p += (od + ev) if swap else (ev + od)
    return np.array(p)


_CONSTS = None


def _static_consts():
    """Input-independent per-core constants (tables, masks, key orders)."""
    global _CONSTS
    if _CONSTS is not None:
        return _CONSTS
    invf = THETA ** (-np.arange(32, dtype=np.float64) / 32)

    def tables(pos):
        ang = pos[None, :] * invf[:, None]
        cos, sin = np.cos(ang), np.sin(ang)
        c2 = np.tile(cos, (4, 1)).astype(np.float32)
        s2 = np.tile(np.vstack([-sin, sin]), (2, 1)).astype(np.float32)
        return c2, s2

    per_j = []
    for j in range(2):
        keypos = np.concatenate(
            [np.arange(QB * q, QB * (q + 1)) for q in KEYORDER[j]])
        qsel = keypos[:TQ]          # queries = first 1024 permuted keys
        c2k, s2k = tables(keypos.astype(np.float64))
        masks = np.zeros((16 * 128, QB), np.float32)
        for s in range(4):
            seq = _slot_seq(s)
            qpos = keypos[QB * s:QB * (s + 1)]
            for k in range(4):
                c = seq[-4 + k]
                kpos = keypos[128 * c:128 * (c + 1)]
                masks[(4 * s + k) * 128:(4 * s + k + 1) * 128] = (
                    kpos[:, None] <= qpos[None, :]).astype(np.float32)
        per_j.append((keypos, qsel, c2k, s2k,
                      masks.astype(ml_dtypes.bfloat16)))
    _CONSTS = per_j
    return _CONSTS


def _host_prep(x, Wq, Wk, Wv, Wo):
    wqT = _rne12(Wq[_neox_perm(H)].T)
    wqsT = _rne12(Wq[_neox_perm(H, swap=True)].T)
    wkT = _rne12(Wk[_neox_perm(HKV)].T)
    wksT = _rne12(Wk[_neox_perm(HKV, swap=True)].T)
    woT = _rne12(Wo.T)
    wvT = np.zeros((577, 260), np.float32)
    for g in range(HKV):
        wvT[:C, 65 * g:65 * g + 64] = Wv[64 * g:64 * g + 64].T
        wvT[576, 65 * g + 64] = 1.0
    wvT = _rne12(wvT)

    per_j = _static_consts()
    x = _rne12(x)
    ones = np.ones((1, T), np.float32)
    in_maps = []
    core_meta = []
    for b in range(B):
        xbT = x[b].T
        for j in range(2):
            keypos, qsel, c2k, s2k, masks = per_j[j]
            xkT = np.vstack([xbT[:, keypos], ones])
            in_maps.append({
                "xkT": xkT,
                "wqT": wqT, "wqsT": wqsT, "wkT": wkT, "wksT": wksT,
                "wvT": wvT, "woT": woT,
                "c2k": c2k, "s2k": s2k,
                "masks": masks,
            })
            core_meta.append((b, qsel))
    return in_maps, core_meta


def kernel(x, Wq, Wk, Wv, Wo):
    x = np.asarray(x, np.float32)
    Wq = np.asarray(Wq, np.float32)
    Wk = np.asarray(Wk, np.float32)
    Wv = np.asarray(Wv, np.float32)
    Wo = np.asarray(Wo, np.float32)

    from concourse.bass_utils import run_bass_kernel_spmd

    nc = _get_program()
    in_maps, core_meta = _host_prep(x, Wq, Wk, Wv, Wo)
    res = run_bass_kernel_spmd(nc, in_maps, list(range(8)))

    out = np.empty((B, T, C), np.float32)
    for core, (b, qsel) in enumerate(core_meta):
        out[b, qsel, :] = res.results[core]["yT"].T
    return out



# revision 44
# speedup vs baseline: 1.4014x; 1.4014x over previous
"""Trainium2 Bass kernel for CausalSelfAttention (RoPE + GQA), 8-core SPMD.

Sharding: 8 cores = 4 batches x 2 query-halves. Each core owns four
query-256-blocks paired {i, 7-i} so causal work is balanced. Keys are
PERMUTED per core: block order = [own q-blocks (desc causal depth), then
remaining blocks ascending]. Slot s's key chunks occupy the static range
starting at chunk 2s; its diagonal chunks are 2s..2s+1 (emitted last in
each slot so one bf16 mask multiply covers them), and the first 1024 key
columns ARE the core's queries -- Q-projection re-reads the same xkT
input tiles and the K RoPE tables double as Q tables. Every core runs an
identical instruction stream; all variation is input data.

Device pipeline per core (vs the 236us baseline):
  * RoPE pair-swap via a single 128x128 permutation matmul on the raw
    projection (PSUM -> bf16 SBUF copy -> perm matmul) instead of a
    second full 5-chunk projection: 6 instead of 10 matmuls per chunk.
  * P.V flipped: out[q, d] with queries on partitions (65-row moving V in
    bf16) instead of out[d+1, q] with 256-row moving P -- half the PE
    rows; softmax denominator comes per-partition so the divide is a
    cheap tensor_scalar; a bf16 PE transpose restores [d, q] for O-proj.
  * Causal padding trimmed 40 -> 36 key chunks (slots 2,3 drop fully
    masked chunks; 2-chunk diagonal groups).
  * O-proj flipped to out[q, m] and DMA'd to DRAM straight from PSUM.
  * exp stays on ACT (~86us floor); V copies / divides / y copies are
    spread over DVE+Pool; wavefront schedule interleaves attention slots
    with K/V projection windows (slot-0 split in two passes with an SBUF
    spill) so ACT work isn't back-loaded.
"""
import sys

sys.path.insert(0, "/opt/trn_rl_repo")

import numpy as np
import ml_dtypes

B, T, C = 4, 2048, 576
H, HKV, D = 9, 3, 64
THETA = 10000.0
QB = 256                      # query block
TQ = 1024                     # queries per core
QBLOCKS = [[7, 5, 2, 0], [6, 4, 3, 1]]   # q-256-block ids per half j
KEYORDER = [[7, 5, 2, 0, 1, 3, 4, 6], [6, 4, 3, 1, 0, 2, 5, 7]]
CCX = [(0, 128), (128, 128), (256, 128), (384, 128), (512, 65)]   # x chunks (577 rows incl ones)
CCQ = [(0, 128), (128, 128), (256, 128), (384, 128), (512, 64)]   # 576-row chunks
MM = [(0, 128), (128, 128), (256, 128), (384, 128), (512, 64)]    # output-dim chunks of 576

# per-slot key-chunk groups (exp granularity); last group of each slot is
# the masked one (diagonal chunks 2s, 2s+1 emitted last). Slot s spans the
# static chunk range [2s, 2s + pad_s) with pad = [16, 12, 8, 4]: the
# per-slot max of the two query-half profiles ([16,12,6,2] for blocks
# [7,5,2,0] and [14,10,8,4] for [6,4,3,1]) -- the host-side mask data
# resolves which chunks are visible per core.
SLOT_GROUPS = [
    [(2, 3, 4, 5), (6, 7, 8, 9), (10, 11, 12, 13), (14, 15, 0, 1)],
    [(4, 5, 6, 7), (8, 9, 10, 11), (12, 13, 2, 3)],
    [(6, 7, 8, 9), (10, 11, 4, 5)],
    [(8, 9, 6, 7)],
]
MASK_W = [4, 4, 4, 4]                      # masked-group widths (chunks)
MASK_OFF = [0, 4, 8, 12]                   # mask-chunk offset per slot
N_MASK = 16

_PROG = None


def _rne12(x):
    """Round fp32 to f32r (RNE, drop 12 mantissa bits) -- matches TRN2."""
    b = np.ascontiguousarray(x, np.float32).view(np.uint32).astype(np.uint64)
    lsb = (b >> np.uint64(12)) & np.uint64(1)
    r = (b + np.uint64(2047) + lsb) >> np.uint64(12) << np.uint64(12)
    return (r & np.uint64(0xFFFFFFFF)).astype(np.uint32).view(np.float32)


def _build_program(ablate=()):
    import concourse.bacc as bacc
    import concourse.mybir as mybir
    import concourse.tile as tile

    dt = mybir.dt
    f32, f32r, bf16 = dt.float32, dt.float32r, dt.bfloat16
    AF = mybir.ActivationFunctionType

    nc = bacc.Bacc("TRN2", target_bir_lowering=False, debug=False, num_devices=8)

    def inp(name, shape, d=f32):
        return nc.declare_dram_parameter(name, shape, d, isOutput=False)

    xkT = inp("xkT", [577, T], f32r)
    wqT = inp("wqT", [C, C], f32r)
    wkT = inp("wkT", [C, HKV * D], f32r)
    wvT = inp("wvT", [577, 260], f32r)
    woT = inp("woT", [C, C], bf16)
    c2k = inp("c2k", [128, T], bf16)
    s2k = inp("s2k", [128, T])
    masksp = inp("masks", [N_MASK * 128, QB], bf16)
    permp = inp("perm", [128, 128], bf16)
    identp = inp("ident", [128, 128], bf16)
    yT = nc.declare_dram_parameter("yT", [TQ, C], f32, isOutput=True)

    with tile.TileContext(nc) as tc:
        with (
            tc.tile_pool(name="const", bufs=1) as cp,
            tc.tile_pool(name="xw", bufs=4) as xwp,          # x window tiles
            tc.tile_pool(name="qraw", bufs=2) as qrp,
            tc.tile_pool(name="rope", bufs=2) as rp,
            tc.tile_pool(name="pb", bufs=3) as pbp,
            tc.tile_pool(name="ysb", bufs=4) as ysp,
            tc.tile_pool(name="small", bufs=2) as smp,
            # PSUM: scores 2x2 banks + y accum 1 + transpose 1 = 6; the
            # projection pools (psA/psB or psR) use the remaining 2.
            tc.tile_pool(name="psS", bufs=2, space="PSUM") as psS,
            tc.tile_pool(name="psY", bufs=1, space="PSUM") as psY,
            tc.tile_pool(name="psT", bufs=1, space="PSUM") as psT,
            tc.tile_pool(name="psA", bufs=1, space="PSUM") as psA,
            tc.tile_pool(name="psB", bufs=1, space="PSUM") as psB,
        ):
            # ---------------- const loads ------------------------------
            # Two HWDGE queues: SP carries the x windows (critical path to
            # the first matmuls), the ACT queue carries weights/tables/masks
            # in consumer order, so startup is not serialized on one queue.
            def load_w(pool, param, chunks, cols, tag, d=f32r, eng=None):
                """Load a row-chunked weight as ONE tile via two batched
                DMAs (4 full 128-row chunks + the partial tail chunk) --
                each HWDGE queue issue costs ~0.7us of SEQ time, so DMA
                count matters more than transfer size here."""
                eng = eng or nc.sync
                nch = len(chunks)
                t = pool.tile([128, nch * cols], d, tag=tag, name=tag)
                nf = nch - 1
                eng.dma_start(
                    t[:, 0:nf * cols].rearrange("b (a c) -> b a c", a=nf),
                    param[0:128 * nf, :].rearrange("(a b) c -> b a c", a=nf))
                k0, kl = chunks[-1]
                eng.dma_start(t[:kl, nf * cols:], param[k0:k0 + kl, :])
                return [t[:, i * cols:(i + 1) * cols] for i in range(nch)]

            def load_xwin(w, eng=None):
                eng = eng or nc.sync
                t = xwp.tile([128, 5 * 512], f32r, tag="xk", name=f"xw{w}")
                eng.dma_start(
                    t[:, 0:4 * 512].rearrange("b (a c) -> b a c", a=4),
                    xkT[0:512, 512 * w:512 * (w + 1)]
                    .rearrange("(a b) c -> b a c", a=4))
                eng.dma_start(t[:65, 4 * 512:],
                              xkT[512:577, 512 * w:512 * (w + 1)])
                return [t[:, i * 512:(i + 1) * 512] for i in range(5)]

            # first window + wq load per-chunk so the first projection
            # matmuls start as soon as chunk 0 lands, not after the batch
            def load_xwin1():
                t = xwp.tile([128, 5 * 512], f32r, tag="xk", name="xw1")
                for i, (k0, kl) in enumerate(CCX):
                    nc.sync.dma_start(t[:kl, 512 * i:512 * (i + 1)],
                                      xkT[k0:k0 + kl, 512:1024])
                return [t[:, i * 512:(i + 1) * 512] for i in range(5)]

            def load_wq():
                t = cp.tile([128, 5 * C], f32r, tag="wq", name="wq")
                for i, (k0, kl) in enumerate(CCQ):
                    nc.scalar.dma_start(t[:kl, C * i:C * (i + 1)],
                                        wqT[k0:k0 + kl, :])
                return [t[:, i * C:(i + 1) * C] for i in range(5)]

            xw1 = load_xwin1()
            wq_r = load_wq()
            c2k_t = cp.tile([128, T], bf16, tag="c2k", name="c2k")
            s2k_t = cp.tile([128, T], f32, tag="s2k", name="s2k")
            perm_t = cp.tile([128, 128], bf16, tag="perm", name="perm")
            ident_t = cp.tile([128, 128], bf16, tag="ident", name="ident")
            nc.scalar.dma_start(perm_t[:], permp[:])
            nc.scalar.dma_start(c2k_t[:], c2k[:])
            nc.scalar.dma_start(s2k_t[:], s2k[:])
            xw2 = load_xwin(2, eng=nc.scalar)
            xw0 = load_xwin(0)
            xw3 = load_xwin(3)
            wk_r = load_w(cp, wkT, CCQ, HKV * D, "wk", eng=nc.scalar)
            wv_r = load_w(cp, wvT, CCX, 260, "wv", eng=nc.scalar)
            m_b = cp.tile([128, N_MASK * QB], bf16, tag="masks", name="masks")
            nc.scalar.dma_start(
                m_b[:, :].rearrange("b (a c) -> b a c", a=N_MASK),
                masksp[:, :].rearrange("(a b) c -> b a c", a=N_MASK))
            nc.scalar.dma_start(ident_t[:], identp[:])
            wo_r = load_w(cp, woT, MM, C, "wo", d=bf16, eng=nc.scalar)

            # persistent projection outputs
            kt_h = [cp.tile([64, T], f32r, tag=f"kt{g}", name=f"kt{g}")
                    for g in range(HKV)]
            qth = [cp.tile([64, TQ], f32r, tag=f"qth{h}", name=f"qth{h}")
                   for h in range(H)]
            v_t = [cp.tile([128, 260], bf16, tag=f"v{c}", name=f"v{c}")
                   for c in range(16)]
            ypr = [cp.tile([128, TQ], bf16, tag=f"ypr{p}", name=f"ypr{p}")
                   for p in range(5)]
            spill_t = {}

            def spill_tile(s_, g):
                if (s_, g) not in spill_t:
                    spill_t[(s_, g)] = cp.tile(
                        [128, 390], f32, tag=f"ysp{s_}_{g}", name=f"ysp{s_}_{g}")
                return spill_t[(s_, g)]

            # ---------------- rope: proj + perm-matmul + combine -------
            def rope_chunk(xr, w_r, mc0, mrows, cols0, dsts, kside=False,
                           act_copy=False):
                """project chunk -> rope -> dsts[bi][0:64, cols0:cols0+512].
                K-side SBUF-only work (t1 mul, adds) goes to the otherwise
                idle Pool engine; PSUM-reading ops must stay on DVE/ACT.
                act_copy routes the PSUM drain to ACT -- used in sections
                where no exp stream is running yet."""
                ps = psA.tile([128, 512], f32, tag="pja", name="pja")
                for ci, (k0, kl) in enumerate(CCQ):
                    nc.tensor.matmul(ps[:mrows, :],
                                     w_r[ci][:kl, mc0:mc0 + mrows],
                                     xr[ci][:kl, :],
                                     start=(ci == 0), stop=(ci == 4))
                qraw = qrp.tile([128, 512], bf16, tag="qraw", name="qraw")
                if act_copy:
                    nc.scalar.activation(qraw[:mrows, :], ps[:mrows, :], AF.Copy)
                else:
                    nc.vector.tensor_copy(qraw[:mrows, :], ps[:mrows, :])
                ps2 = psB.tile([128, 512], f32, tag="pjb", name="pjb")
                nc.tensor.matmul(ps2[:mrows, :], perm_t[:mrows, :mrows],
                                 qraw[:mrows, :], start=True, stop=True)
                t1 = rp.tile([128, 512], bf16, tag="rope1", name="rope1")
                t2 = rp.tile([128, 512], f32, tag="rope2", name="rope2")
                t1eng = nc.gpsimd if kside else nc.vector
                t1eng.tensor_mul(t1[:mrows, :], qraw[:mrows, :],
                                 c2k_t[:mrows, cols0:cols0 + 512])
                nc.vector.tensor_mul(t2[:mrows, :], ps2[:mrows, :],
                                     s2k_t[:mrows, cols0:cols0 + 512])
                for bi, dt_ in enumerate(dsts):
                    eng = nc.gpsimd if (kside or bi % 2) else nc.vector
                    eng.tensor_add(dt_[0:64, cols0:cols0 + 512],
                                   t1[64 * bi:64 * bi + 64, :],
                                   t2[64 * bi:64 * bi + 64, :])

            def qproj_units(w, xr):
                units = []
                for m, (mc0, mrows) in enumerate(MM):
                    dsts = ([qth[2 * m], qth[2 * m + 1]] if m < 4 else [qth[8]])
                    units.append(lambda mc0=mc0, mrows=mrows, dsts=dsts:
                                 rope_chunk(xr, wq_r, mc0, mrows, 512 * w,
                                            dsts))
                return units

            def vproj_one(w, xr, ti, act_copy=False):
                t_ = 4 * w + ti
                vpool = psA if ti % 2 == 0 else psB
                vtag = "pja" if ti % 2 == 0 else "pjb"
                ps = vpool.tile([128, 512], f32, tag=vtag, name=vtag)
                for ci, (k0, kl) in enumerate(CCX):
                    nc.tensor.matmul(ps[:, :260],
                                     xr[ci][:kl, 128 * ti:128 * (ti + 1)],
                                     wv_r[ci][:kl, :],
                                     start=(ci == 0), stop=(ci == 4))
                if act_copy:
                    nc.scalar.activation(v_t[t_][:], ps[:, :260], AF.Copy)
                else:
                    nc.vector.tensor_copy(v_t[t_][:], ps[:, :260])

            def kvproj_units(w, xr, act_copy=False):
                units = []
                for mi, (mc0, mrows) in enumerate([(0, 128), (128, 64)]):
                    dsts = [kt_h[0], kt_h[1]] if mi == 0 else [kt_h[2]]
                    units.append(lambda mc0=mc0, mrows=mrows, dsts=dsts:
                                 rope_chunk(xr, wk_r, mc0, mrows, 512 * w,
                                            dsts, kside=True,
                                            act_copy=act_copy))
                for ti in range(4):
                    units.append(lambda ti=ti: vproj_one(w, xr, ti,
                                                         act_copy=act_copy))
                return units

            def qproj_win(w, xr):
                for u in qproj_units(w, xr):
                    u()

            def kvproj_win(w, xr):
                for u in kvproj_units(w, xr):
                    u()

            # ---------------- attention --------------------------------
            def attn_groups(s, groups, spill=None, first=True, last=True,
                            fillers=()):
                """Process score/PV groups for slot s. If spill is given and
                last=False, accumulate into psY then copy to the slot's spill
                tiles (partial pass A); if spill is given and first=False,
                merge the spill into the final result.

                Emission is software-pipelined: the PV matmuls for step i are
                emitted after the S matmuls of step i+1 so the in-order PE
                queue never waits on the exp (ACT) of the current step.
                `fillers` are independent PE work units (projection chunks,
                O-proj blocks) interleaved between steps to cover the
                exp-bound deficit of the attention pipeline."""
                nchunks = sum(len(gr) for gr in groups)
                steps = []          # (g, hl, sc, grp, ci0)
                for g in range(HKV):
                    for hl in range(3):
                        ci0 = 0
                        for sc, grp in enumerate(groups):
                            steps.append((g, hl, sc, grp, ci0))
                            ci0 += len(grp)

                def emit_s(st):
                    g, hl, sc, grp, ci0 = st
                    h = 3 * g + hl
                    sp = psS.tile([128, 1024], f32, tag="scores", name="scores")
                    for i, c_ in enumerate(grp):
                        nc.tensor.matmul(
                            sp[:, QB * i:QB * (i + 1)],
                            kt_h[g][0:64, 128 * c_:128 * (c_ + 1)],
                            qth[h][0:64, QB * s:QB * (s + 1)],
                            start=True, stop=True)
                    wgrp = len(grp)
                    p_b = pbp.tile([128, 1024], bf16, tag="p", name="p")
                    nc.scalar.activation(p_b[:, :QB * wgrp], sp[:, :QB * wgrp],
                                         AF.Exp, scale=0.125)
                    if last and sc == len(groups) - 1:
                        mo = QB * MASK_OFF[s]
                        nc.vector.tensor_mul(
                            p_b[:, :QB * wgrp], p_b[:, :QB * wgrp],
                            m_b[:, mo:mo + QB * wgrp])
                    return p_b

                def emit_pv(st, p_b, yt2):
                    # one accumulation group per PSUM bank: start on the very
                    # first matmul into yt2, stop on the very last; interior
                    # regions are zeroed on first touch (pending-zero).
                    g, hl, sc, grp, ci0 = st
                    for i, c_ in enumerate(grp):
                        for qb in range(2):
                            nc.tensor.matmul(
                                yt2[:, 130 * hl + 65 * qb:
                                    130 * hl + 65 * qb + 65],
                                p_b[:, QB * i + 128 * qb:
                                    QB * i + 128 * qb + 128],
                                v_t[c_][:, 65 * g:65 * g + 65],
                                start=(hl == 0 and ci0 + i == 0 and qb == 0),
                                stop=(hl == 2 and ci0 + i == nchunks - 1
                                      and qb == 1))

                def drain(g, yt2):
                    if spill is not None and not last:
                        nc.vector.tensor_copy(spill_tile(s, g)[:], yt2[:])
                        return
                    merged = yt2
                    merged_sbuf = False
                    if spill is not None and not first:
                        msb = smp.tile([128, 390], f32, tag="merged",
                                       name="merged")
                        nc.vector.tensor_add(msb[:], yt2[:], spill_tile(s, g)[:])
                        merged = msb
                        merged_sbuf = True
                    rcp = smp.tile([128, 6], f32, tag="rcp", name="rcp")
                    nc.vector.reciprocal(rcp[:], merged[:, 64::65])
                    # SBUF-resident merge (slot-0 pass B) can divide on Pool
                    ts_eng = nc.gpsimd if merged_sbuf else nc.vector
                    for hl in range(3):
                        h = 3 * g + hl
                        for qb in range(2):
                            ysb = _ysb_for(s, qb)
                            ts_eng.tensor_scalar_mul(
                                ysb[:, 64 * h:64 * h + 64],
                                merged[:, 130 * hl + 65 * qb:
                                       130 * hl + 65 * qb + 64],
                                rcp[:, 2 * hl + qb:2 * hl + qb + 1])

                fillers = list(fillers)
                stride = max(1, (len(steps) + len(fillers)) // (len(fillers) + 1)) \
                    if fillers else 0
                yt2_of = {}
                pend = None         # (step, p_b)
                for si, st in enumerate(steps):
                    g = st[0]
                    if g not in yt2_of:
                        # drain previous group before its bank is reused
                        if pend is not None and pend[0][0] != g:
                            emit_pv(pend[0], pend[1], yt2_of[pend[0][0]])
                            pend = None
                        if g - 1 in yt2_of:
                            drain(g - 1, yt2_of[g - 1])
                        yt2_of[g] = psY.tile([128, 390], f32, tag="yt2",
                                             name="yt2")
                    p_b = emit_s(st)
                    if pend is not None:
                        emit_pv(pend[0], pend[1], yt2_of[pend[0][0]])
                    pend = (st, p_b)
                    if fillers and si % stride == stride - 1:
                        fillers.pop(0)()
                if pend is not None:
                    emit_pv(pend[0], pend[1], yt2_of[pend[0][0]])
                drain(HKV - 1, yt2_of[HKV - 1])
                for f in fillers:
                    f()

            _ysb_cache = {}

            def _ysb_for(s, qb):
                key = (s, qb)
                if key not in _ysb_cache:
                    _ysb_cache[key] = ysp.tile([128, 576], bf16, tag="ysb",
                                               name=f"ysb{s}_{qb}")
                return _ysb_cache[key]

            def finish_qb(s, qb):
                """transpose ysb -> ypr for one query-128-block of slot s."""
                ysb = _ysb_cache.pop((s, qb))
                tpp = psT.tile([128, 640], bf16, tag="tpp", name="tpp")
                for p, (pc0, pl) in enumerate(MM):
                    nc.tensor.transpose(tpp[:pl, 128 * p:128 * (p + 1)],
                                        ysb[:, pc0:pc0 + pl],
                                        ident_t[:, :])
                for p, (pc0, pl) in enumerate(MM):
                    nc.vector.tensor_copy(
                        ypr[p][:pl, QB * s + 128 * qb:QB * s + 128 * qb + 128],
                        tpp[:pl, 128 * p:128 * (p + 1)])

            def attn_finish(s):
                for qb in range(2):
                    finish_qb(s, qb)

            def attn_slot(s, fillers=()):
                attn_groups(s, SLOT_GROUPS[s], fillers=fillers)
                attn_finish(s)

            # ---------------- output projection ------------------------
            def oproj_qi(qi, act_copy=False):
                    psr = psA.tile([128, 512], f32, tag="pja", name="pja")
                    for p, (pc0, pl) in enumerate(MM):
                        nc.tensor.matmul(psr[:, :],
                                         ypr[p][:pl, 128 * qi:128 * (qi + 1)],
                                         wo_r[p][:pl, 0:512],
                                         start=(p == 0), stop=(p == 4))
                    psr2 = psB.tile([128, 512], f32, tag="pjb", name="pjb")
                    for p, (pc0, pl) in enumerate(MM):
                        nc.tensor.matmul(psr2[:, :64],
                                         ypr[p][:pl, 128 * qi:128 * (qi + 1)],
                                         wo_r[p][:pl, 512:576],
                                         start=(p == 0), stop=(p == 4))
                    ost = ysp.tile([128, 576], f32, tag="ost", name="ost")
                    if act_copy:
                        nc.scalar.activation(ost[:, 0:512], psr[:], AF.Copy)
                    else:
                        nc.vector.tensor_copy(ost[:, 0:512], psr[:])
                    nc.vector.tensor_copy(ost[:, 512:576], psr2[:, :64])
                    dma_eng = nc.scalar if qi % 2 else nc.sync
                    dma_eng.dma_start(yT[128 * qi:128 * (qi + 1), :], ost[:])

            # ---------------- schedule ---------------------------------
            # KV windows run in order 1,2,0,3 so slots 3 and 2 (which need
            # chunks 4-11) unlock after two windows and the exp stream on
            # ACT starts early. Later projection windows and O-proj blocks
            # ride as fillers inside the exp-bound attention sections, and
            # slots 0 and 1 are split in two passes (SBUF spill) so their
            # exp work spreads across the whole kernel instead of the tail.
            attn = "attn" not in ablate
            opj = "oproj" not in ablate and attn
            for u in qproj_units(1, xw1):        # queries 512:1024
                u()
            for u in kvproj_units(1, xw1, act_copy=True):   # key chunks 4-7
                u()
            for u in kvproj_units(2, xw2, act_copy=True):   # key chunks 8-11
                u()
            if attn:
                attn_slot(3, fillers=qproj_units(0, xw0))   # chunks 6-9
                attn_slot(2, fillers=kvproj_units(0, xw0))  # chunks 4-11
                # slot 1 pass A: first two groups (chunks 4-11)
                attn_groups(1, SLOT_GROUPS[1][:2], spill=True, last=False,
                            fillers=[lambda: oproj_qi(6), lambda: oproj_qi(7)]
                            if opj else ())
            if attn:
                # slot 0 pass A: first two groups (chunks 2-9)
                attn_groups(0, SLOT_GROUPS[0][:2], spill=True, last=False,
                            fillers=kvproj_units(3, xw3))   # chunks 12-15
                # slot 1 pass B: last group (chunks 12,13 + diag 2,3)
                attn_groups(1, SLOT_GROUPS[1][2:], spill=True, first=False,
                            fillers=[lambda: oproj_qi(4), lambda: oproj_qi(5)]
                            if opj else ())
                attn_finish(1)
                # slot 0 pass B: last two groups (chunks 10-15, 0-1)
                attn_groups(0, SLOT_GROUPS[0][2:], spill=True, first=False,
                            fillers=[lambda: oproj_qi(2), lambda: oproj_qi(3)]
                            if opj else ())
                finish_qb(0, 0)
                if "oproj" not in ablate:
                    oproj_qi(0, act_copy=True)
                finish_qb(0, 1)
                if "oproj" not in ablate:
                    oproj_qi(1, act_copy=True)

    nc.compile()
    return nc


def _get_program():
    global _PROG
    if _PROG is None:
        _PROG = _build_program()
    return _PROG


def _neox_perm(nheads, swap=False):
    p = []
    for h in range(nheads):
        ev = [64 * h + 2 * j for j in range(32)]
        od = [64 * h + 2 * j + 1 for j in range(32)]
        p += (od + ev) if swap else (ev + od)
    return np.array(p)


_CONSTS = None


def _static_consts():
    """Input-independent per-core constants (tables, masks, key orders)."""
    global _CONSTS
    if _CONSTS is not None:
        return _CONSTS
    invf = THETA ** (-np.arange(32, dtype=np.float64) / 32)

    def tables(pos):
        ang = pos[None, :] * invf[:, None]
        cos, sin = np.cos(ang), np.sin(ang)
        c2 = np.tile(cos, (4, 1)).astype(np.float32)
        s2 = np.tile(np.vstack([-sin, sin]), (2, 1)).astype(np.float32)
        return c2, s2

    per_j = []
    for j in range(2):
        keypos = np.concatenate(
            [np.arange(QB * q, QB * (q + 1)) for q in KEYORDER[j]])
        qsel = keypos[:TQ]          # queries = first 1024 permuted keys
        c2k, s2k = tables(keypos.astype(np.float64))
        masks = np.zeros((N_MASK * 128, QB), np.float32)
        for s in range(4):
            grp = SLOT_GROUPS[s][-1]
            qpos = keypos[QB * s:QB * (s + 1)]
            for k, c in enumerate(grp):
                kpos = keypos[128 * c:128 * (c + 1)]
                mi = MASK_OFF[s] + k
                masks[mi * 128:(mi + 1) * 128] = (
                    kpos[:, None] <= qpos[None, :]).astype(np.float32)
        per_j.append((keypos, qsel,
                      c2k.astype(ml_dtypes.bfloat16), s2k,
                      masks.astype(ml_dtypes.bfloat16)))
    _CONSTS = per_j
    return _CONSTS


def _perm_matrix():
    """128x128 block-diag pair-swap (per 64 rows: swap 32-halves)."""
    p = np.zeros((128, 128), np.float32)
    for b in range(2):
        for i in range(32):
            p[64 * b + 32 + i, 64 * b + i] = 1.0        # out[m]=in[swap(m)]
            p[64 * b + i, 64 * b + 32 + i] = 1.0
    return p.astype(ml_dtypes.bfloat16)


def _host_prep(x, Wq, Wk, Wv, Wo):
    wqT = _rne12(Wq[_neox_perm(H)].T)
    wkT = _rne12(Wk[_neox_perm(HKV)].T)
    woT = Wo.T.astype(ml_dtypes.bfloat16)
    wvT = np.zeros((577, 260), np.float32)
    for g in range(HKV):
        wvT[:C, 65 * g:65 * g + 64] = Wv[64 * g:64 * g + 64].T
        wvT[576, 65 * g + 64] = 1.0
    wvT = _rne12(wvT)
    perm = _perm_matrix()
    ident = np.eye(128, dtype=np.float32).astype(ml_dtypes.bfloat16)

    per_j = _static_consts()
    x = _rne12(x)
    ones = np.ones((1, T), np.float32)
    in_maps = []
    core_meta = []
    for b in range(B):
        xbT = x[b].T
        for j in range(2):
            keypos, qsel, c2k, s2k, masks = per_j[j]
            xkT = np.vstack([xbT[:, keypos], ones])
            in_maps.append({
                "xkT": xkT,
                "wqT": wqT, "wkT": wkT, "wvT": wvT, "woT": woT,
                "c2k": c2k, "s2k": s2k,
                "masks": masks, "perm": perm, "ident": ident,
            })
            core_meta.append((b, qsel))
    return in_maps, core_meta


def kernel(x, Wq, Wk, Wv, Wo):
    x = np.asarray(x, np.float32)
    Wq = np.asarray(Wq, np.float32)
    Wk = np.asarray(Wk, np.float32)
    Wv = np.asarray(Wv, np.float32)
    Wo = np.asarray(Wo, np.float32)

    from concourse.bass_utils import run_bass_kernel_spmd

    nc = _get_program()
    in_maps, core_meta = _host_prep(x, Wq, Wk, Wv, Wo)
    res = run_bass_kernel_spmd(nc, in_maps, list(range(8)))

    out = np.empty((B, T, C), np.float32)
    for core, (b, qsel) in enumerate(core_meta):
        out[b, qsel, :] = res.results[core]["yT"]
    return out


# revision 53
# speedup vs baseline: 1.4752x; 1.0527x over previous
"""Trainium2 Bass kernel for CausalSelfAttention (RoPE + GQA), 8-core SPMD.

Sharding: 8 cores = 4 batches x 2 query-halves. Each core owns four
query-256-blocks paired {i, 7-i} so causal work is balanced. Keys are
PERMUTED per core: block order = [own q-blocks (desc causal depth), then
remaining blocks ascending]. Slot s's key chunks occupy the static range
starting at chunk 2s; its diagonal chunks are 2s..2s+1 (emitted last in
each slot so one bf16 mask multiply covers them), and the first 1024 key
columns ARE the core's queries -- Q-projection re-reads the same xkT
input tiles and the K RoPE tables double as Q tables. Every core runs an
identical instruction stream; all variation is input data.

Device pipeline per core (160us vs the 236us baseline):
  * RoPE pair-swap via a single 128x128 permutation matmul on the raw
    projection (PSUM -> bf16 SBUF copy -> perm matmul) instead of a
    second full 5-chunk projection: 6 instead of 10 matmuls per chunk.
  * P.V flipped: out[q, d] with queries on partitions (65-row moving V in
    bf16) instead of out[d+1, q] with 256-row moving P -- half the PE
    rows; softmax denominator comes per-partition so the divide is a
    cheap tensor_scalar; a bf16 PE transpose restores [d, q] for O-proj,
    whose out[q, m] form then streams bf16 woT as the moving operand.
  * One PSUM accumulation group per yt2 bank (start on first PV matmul,
    stop on last; interior regions zero on first touch).
  * exp on ACT is the co-critical ~93us floor next to PE's ~102us: the
    wavefront schedule (KV windows in order 1,2,0,3) starts the exp
    stream early, slots 0 and 1 are split in two passes (SBUF spill +
    merge) so their exp spreads forward, and later projection windows /
    O-proj blocks ride as fillers inside the exp-bound attention
    sections; PV matmuls are emitted one step behind the next S group so
    the in-order PE queue never waits on the current exp.
  * Weight/mask/x loads are batched multi-level-AP DMAs split across the
    SP and ACT HWDGE queues in consumer order (queue issue costs
    ~0.7us each); first window + wq stay per-chunk so matmul 0 starts
    at ~4.5us.
  * bf16 for everything off the f32r spine (P, V, masks, q/k rope
    outputs, rope tables, Wo, y) -- rel err ~6e-3 vs the 2e-2 gate.
"""
import sys

sys.path.insert(0, "/opt/trn_rl_repo")

import numpy as np
import ml_dtypes

B, T, C = 4, 2048, 576
H, HKV, D = 9, 3, 64
THETA = 10000.0
QB = 256                      # query block
TQ = 1024                     # queries per core
QBLOCKS = [[7, 5, 2, 0], [6, 4, 3, 1]]   # q-256-block ids per half j
KEYORDER = [[7, 5, 2, 0, 1, 3, 4, 6], [6, 4, 3, 1, 0, 2, 5, 7]]
CCX = [(0, 128), (128, 128), (256, 128), (384, 128), (512, 65)]   # x chunks (577 rows incl ones)
CCQ = [(0, 128), (128, 128), (256, 128), (384, 128), (512, 64)]   # 576-row chunks
MM = [(0, 128), (128, 128), (256, 128), (384, 128), (512, 64)]    # output-dim chunks of 576

# per-slot key-chunk groups (exp granularity); last group of each slot is
# the masked one (diagonal chunks 2s, 2s+1 emitted last). Slot s spans the
# static chunk range [2s, 2s + pad_s) with pad = [16, 12, 8, 4]: the
# per-slot max of the two query-half profiles ([16,12,6,2] for blocks
# [7,5,2,0] and [14,10,8,4] for [6,4,3,1]) -- the host-side mask data
# resolves which chunks are visible per core.
SLOT_GROUPS = [
    [(2, 3, 4, 5), (6, 7, 8, 9), (10, 11, 12, 13), (14, 15, 0, 1)],
    [(4, 5, 6, 7), (8, 9, 10, 11), (12, 13, 2, 3)],
    [(6, 7, 8, 9), (10, 11, 4, 5)],
    [(8, 9, 6, 7)],
]
MASK_W = [4, 4, 4, 4]                      # masked-group widths (chunks)
MASK_OFF = [0, 4, 8, 12]                   # mask-chunk offset per slot
N_MASK = 16

_PROG = None


def _rne12(x):
    """Round fp32 to f32r (RNE, drop 12 mantissa bits) -- matches TRN2."""
    b = np.ascontiguousarray(x, np.float32).view(np.uint32).astype(np.uint64)
    lsb = (b >> np.uint64(12)) & np.uint64(1)
    r = (b + np.uint64(2047) + lsb) >> np.uint64(12) << np.uint64(12)
    return (r & np.uint64(0xFFFFFFFF)).astype(np.uint32).view(np.float32)


def _build_program(ablate=()):
    import concourse.bacc as bacc
    import concourse.mybir as mybir
    import concourse.tile as tile

    dt = mybir.dt
    f32, f32r, bf16 = dt.float32, dt.float32r, dt.bfloat16
    AF = mybir.ActivationFunctionType

    nc = bacc.Bacc("TRN2", target_bir_lowering=False, debug=False, num_devices=8)

    def inp(name, shape, d=f32):
        return nc.declare_dram_parameter(name, shape, d, isOutput=False)

    xkT = inp("xkT", [577, T], f32r)
    wqT = inp("wqT", [C, C], f32r)
    wkT = inp("wkT", [C, HKV * D], f32r)
    wvT = inp("wvT", [577, 260], f32r)
    woT = inp("woT", [C, C], bf16)
    c2k = inp("c2k", [128, T], bf16)
    s2k = inp("s2k", [128, T], bf16)
    masksp = inp("masks", [N_MASK * 128, QB], bf16)
    permp = inp("perm", [128, 128], bf16)
    identp = inp("ident", [128, 128], bf16)
    yT = nc.declare_dram_parameter("yT", [TQ, C], f32, isOutput=True)

    with tile.TileContext(nc) as tc:
        with (
            tc.tile_pool(name="const", bufs=1) as cp,
            tc.tile_pool(name="xw", bufs=4) as xwp,          # x window tiles
            tc.tile_pool(name="qraw", bufs=3) as qrp,
            tc.tile_pool(name="rope", bufs=2) as rp,
            tc.tile_pool(name="pb", bufs=5) as pbp,
            tc.tile_pool(name="ysb", bufs=6) as ysp,
            tc.tile_pool(name="small", bufs=2) as smp,
            # PSUM: scores 2x2 banks + y accum 1 + transpose 1 = 6; the
            # projection pools (psA/psB or psR) use the remaining 2.
            tc.tile_pool(name="psS", bufs=2, space="PSUM") as psS,
            tc.tile_pool(name="psY", bufs=1, space="PSUM") as psY,
            tc.tile_pool(name="psT", bufs=1, space="PSUM") as psT,
            tc.tile_pool(name="psA", bufs=1, space="PSUM") as psA,
            tc.tile_pool(name="psB", bufs=1, space="PSUM") as psB,
        ):
            # ---------------- const loads ------------------------------
            # Two HWDGE queues: SP carries the x windows (critical path to
            # the first matmuls), the ACT queue carries weights/tables/masks
            # in consumer order, so startup is not serialized on one queue.
            def load_w(pool, param, chunks, cols, tag, d=f32r, eng=None):
                """Load a row-chunked weight as ONE tile via two batched
                DMAs (4 full 128-row chunks + the partial tail chunk) --
                each HWDGE queue issue costs ~0.7us of SEQ time, so DMA
                count matters more than transfer size here."""
                eng = eng or nc.sync
                nch = len(chunks)
                t = pool.tile([128, nch * cols], d, tag=tag, name=tag)
                nf = nch - 1
                eng.dma_start(
                    t[:, 0:nf * cols].rearrange("b (a c) -> b a c", a=nf),
                    param[0:128 * nf, :].rearrange("(a b) c -> b a c", a=nf))
                k0, kl = chunks[-1]
                eng.dma_start(t[:kl, nf * cols:], param[k0:k0 + kl, :])
                return [t[:, i * cols:(i + 1) * cols] for i in range(nch)]

            def load_xwin(w, eng=None):
                eng = eng or nc.sync
                t = xwp.tile([128, 5 * 512], f32r, tag="xk", name=f"xw{w}")
                eng.dma_start(
                    t[:, 0:4 * 512].rearrange("b (a c) -> b a c", a=4),
                    xkT[0:512, 512 * w:512 * (w + 1)]
                    .rearrange("(a b) c -> b a c", a=4))
                eng.dma_start(t[:65, 4 * 512:],
                              xkT[512:577, 512 * w:512 * (w + 1)])
                return [t[:, i * 512:(i + 1) * 512] for i in range(5)]

            # first window + wq load per-chunk so the first projection
            # matmuls start as soon as chunk 0 lands, not after the batch
            def load_xwin1():
                t = xwp.tile([128, 5 * 512], f32r, tag="xk", name="xw1")
                for i, (k0, kl) in enumerate(CCX):
                    nc.sync.dma_start(t[:kl, 512 * i:512 * (i + 1)],
                                      xkT[k0:k0 + kl, 512:1024])
                return [t[:, i * 512:(i + 1) * 512] for i in range(5)]

            def load_wq():
                t = cp.tile([128, 5 * C], f32r, tag="wq", name="wq")
                for i, (k0, kl) in enumerate(CCQ):
                    nc.scalar.dma_start(t[:kl, C * i:C * (i + 1)],
                                        wqT[k0:k0 + kl, :])
                return [t[:, i * C:(i + 1) * C] for i in range(5)]

            xw1 = load_xwin1()
            wq_r = load_wq()
            c2k_t = cp.tile([128, T], bf16, tag="c2k", name="c2k")
            s2k_t = cp.tile([128, T], bf16, tag="s2k", name="s2k")
            perm_t = cp.tile([128, 128], bf16, tag="perm", name="perm")
            ident_t = cp.tile([128, 128], bf16, tag="ident", name="ident")
            nc.scalar.dma_start(perm_t[:], permp[:])
            nc.scalar.dma_start(c2k_t[:], c2k[:])
            nc.scalar.dma_start(s2k_t[:], s2k[:])
            xw2 = load_xwin(2)
            xw0 = load_xwin(0)
            xw3 = load_xwin(3)
            wk_r = load_w(cp, wkT, CCQ, HKV * D, "wk", eng=nc.scalar)
            wv_r = load_w(cp, wvT, CCX, 260, "wv", eng=nc.scalar)
            m_b = cp.tile([128, N_MASK * QB], bf16, tag="masks", name="masks")
            nc.scalar.dma_start(
                m_b[:, :].rearrange("b (a c) -> b a c", a=N_MASK),
                masksp[:, :].rearrange("(a b) c -> b a c", a=N_MASK))
            nc.scalar.dma_start(ident_t[:], identp[:])
            wo_r = load_w(cp, woT, MM, C, "wo", d=bf16, eng=nc.scalar)

            # persistent projection outputs
            kt_h = [cp.tile([64, T], bf16, tag=f"kt{g}", name=f"kt{g}")
                    for g in range(HKV)]
            qth = [cp.tile([64, TQ], bf16, tag=f"qth{h}", name=f"qth{h}")
                   for h in range(H)]
            v_t = [cp.tile([128, 260], bf16, tag=f"v{c}", name=f"v{c}")
                   for c in range(16)]
            ypr = [cp.tile([128, TQ], bf16, tag=f"ypr{p}", name=f"ypr{p}")
                   for p in range(5)]
            spill_t = {}

            def spill_tile(s_, g):
                if (s_, g) not in spill_t:
                    spill_t[(s_, g)] = cp.tile(
                        [128, 390], f32, tag=f"ysp{s_}_{g}", name=f"ysp{s_}_{g}")
                return spill_t[(s_, g)]

            # ---------------- rope: proj + perm-matmul + combine -------
            def rope_chunk(xr, w_r, mc0, mrows, cols0, dsts, kside=False,
                           act_copy=False):
                """project chunk -> rope -> dsts[bi][0:64, cols0:cols0+512].
                K-side SBUF-only work (t1 mul, adds) goes to the otherwise
                idle Pool engine; PSUM-reading ops must stay on DVE/ACT.
                act_copy routes the PSUM drain to ACT -- used in sections
                where no exp stream is running yet."""
                ps = psA.tile([128, 512], f32, tag="pja", name="pja")
                for ci, (k0, kl) in enumerate(CCQ):
                    nc.tensor.matmul(ps[:mrows, :],
                                     w_r[ci][:kl, mc0:mc0 + mrows],
                                     xr[ci][:kl, :],
                                     start=(ci == 0), stop=(ci == 4))
                qraw = qrp.tile([128, 512], bf16, tag="qraw", name="qraw")
                if act_copy:
                    nc.scalar.activation(qraw[:mrows, :], ps[:mrows, :], AF.Copy)
                else:
                    nc.vector.tensor_copy(qraw[:mrows, :], ps[:mrows, :])
                ps2 = psB.tile([128, 512], f32, tag="pjb", name="pjb")
                nc.tensor.matmul(ps2[:mrows, :], perm_t[:mrows, :mrows],
                                 qraw[:mrows, :], start=True, stop=True)
                t1 = rp.tile([128, 512], bf16, tag="rope1", name="rope1")
                t2 = rp.tile([128, 512], bf16, tag="rope2", name="rope2")
                t1eng = nc.gpsimd if kside else nc.vector
                t1eng.tensor_mul(t1[:mrows, :], qraw[:mrows, :],
                                 c2k_t[:mrows, cols0:cols0 + 512])
                nc.vector.tensor_mul(t2[:mrows, :], ps2[:mrows, :],
                                     s2k_t[:mrows, cols0:cols0 + 512])
                for bi, dt_ in enumerate(dsts):
                    eng = nc.gpsimd if (kside or bi % 2) else nc.vector
                    eng.tensor_add(dt_[0:64, cols0:cols0 + 512],
                                   t1[64 * bi:64 * bi + 64, :],
                                   t2[64 * bi:64 * bi + 64, :])

            def qproj_units(w, xr):
                units = []
                for m, (mc0, mrows) in enumerate(MM):
                    dsts = ([qth[2 * m], qth[2 * m + 1]] if m < 4 else [qth[8]])
                    units.append(lambda mc0=mc0, mrows=mrows, dsts=dsts:
                                 rope_chunk(xr, wq_r, mc0, mrows, 512 * w,
                                            dsts))
                return units

            def vproj_one(w, xr, ti, act_copy=False):
                t_ = 4 * w + ti
                vpool = psA if ti % 2 == 0 else psB
                vtag = "pja" if ti % 2 == 0 else "pjb"
                ps = vpool.tile([128, 512], f32, tag=vtag, name=vtag)
                for ci, (k0, kl) in enumerate(CCX):
                    nc.tensor.matmul(ps[:, :260],
                                     xr[ci][:kl, 128 * ti:128 * (ti + 1)],
                                     wv_r[ci][:kl, :],
                                     start=(ci == 0), stop=(ci == 4))
                if act_copy:
                    nc.scalar.activation(v_t[t_][:], ps[:, :260], AF.Copy)
                else:
                    nc.vector.tensor_copy(v_t[t_][:], ps[:, :260])

            def kvproj_units(w, xr, act_copy=False):
                units = []
                for mi, (mc0, mrows) in enumerate([(0, 128), (128, 64)]):
                    dsts = [kt_h[0], kt_h[1]] if mi == 0 else [kt_h[2]]
                    units.append(lambda mc0=mc0, mrows=mrows, dsts=dsts:
                                 rope_chunk(xr, wk_r, mc0, mrows, 512 * w,
                                            dsts, kside=True,
                                            act_copy=act_copy))
                for ti in range(4):
                    units.append(lambda ti=ti: vproj_one(w, xr, ti))
                return units

            def qproj_win(w, xr):
                for u in qproj_units(w, xr):
                    u()

            def kvproj_win(w, xr):
                for u in kvproj_units(w, xr):
                    u()

            # ---------------- attention --------------------------------
            def attn_groups(s, groups, spill=None, first=True, last=True,
                            fillers=()):
                """Process score/PV groups for slot s. If spill is given and
                last=False, accumulate into psY then copy to the slot's spill
                tiles (partial pass A); if spill is given and first=False,
                merge the spill into the final result.

                Emission is software-pipelined: the PV matmuls for step i are
                emitted after the S matmuls of step i+1 so the in-order PE
                queue never waits on the exp (ACT) of the current step.
                `fillers` are independent PE work units (projection chunks,
                O-proj blocks) interleaved between steps to cover the
                exp-bound deficit of the attention pipeline."""
                nchunks = sum(len(gr) for gr in groups)
                steps = []          # (g, hl, sc, grp, ci0)
                for g in range(HKV):
                    for hl in range(3):
                        ci0 = 0
                        for sc, grp in enumerate(groups):
                            steps.append((g, hl, sc, grp, ci0))
                            ci0 += len(grp)

                def emit_s(st):
                    g, hl, sc, grp, ci0 = st
                    h = 3 * g + hl
                    sp = psS.tile([128, 1024], f32, tag="scores", name="scores")
                    for i, c_ in enumerate(grp):
                        nc.tensor.matmul(
                            sp[:, QB * i:QB * (i + 1)],
                            kt_h[g][0:64, 128 * c_:128 * (c_ + 1)],
                            qth[h][0:64, QB * s:QB * (s + 1)],
                            start=True, stop=True)
                    wgrp = len(grp)
                    p_b = pbp.tile([128, 1024], bf16, tag="p", name="p")
                    nc.scalar.activation(p_b[:, :QB * wgrp], sp[:, :QB * wgrp],
                                         AF.Exp, scale=0.125)
                    if last and sc == len(groups) - 1:
                        mo = QB * MASK_OFF[s]
                        nc.vector.tensor_mul(
                            p_b[:, :QB * wgrp], p_b[:, :QB * wgrp],
                            m_b[:, mo:mo + QB * wgrp])
                    return p_b

                def emit_pv(st, p_b, yt2):
                    # one accumulation group per PSUM bank: start on the very
                    # first matmul into yt2, stop on the very last; interior
                    # regions are zeroed on first touch (pending-zero).
                    g, hl, sc, grp, ci0 = st
                    for i, c_ in enumerate(grp):
                        for qb in range(2):
                            nc.tensor.matmul(
                                yt2[:, 130 * hl + 65 * qb:
                                    130 * hl + 65 * qb + 65],
                                p_b[:, QB * i + 128 * qb:
                                    QB * i + 128 * qb + 128],
                                v_t[c_][:, 65 * g:65 * g + 65],
                                start=(hl == 0 and ci0 + i == 0 and qb == 0),
                                stop=(hl == 2 and ci0 + i == nchunks - 1
                                      and qb == 1))

                def drain(g, yt2):
                    if spill is not None and not last:
                        nc.vector.tensor_copy(spill_tile(s, g)[:], yt2[:])
                        return
                    merged = yt2
                    merged_sbuf = False
                    if spill is not None and not first:
                        msb = smp.tile([128, 390], f32, tag="merged",
                                       name="merged")
                        nc.vector.tensor_add(msb[:], yt2[:], spill_tile(s, g)[:])
                        merged = msb
                        merged_sbuf = True
                    rcp = smp.tile([128, 6], f32, tag="rcp", name="rcp")
                    nc.vector.reciprocal(rcp[:], merged[:, 64::65])
                    # SBUF-resident merge (slot-0 pass B) can divide on Pool
                    ts_eng = nc.gpsimd if merged_sbuf else nc.vector
                    for hl in range(3):
                        h = 3 * g + hl
                        for qb in range(2):
                            ysb = _ysb_for(s, qb)
                            ts_eng.tensor_scalar_mul(
                                ysb[:, 64 * h:64 * h + 64],
                                merged[:, 130 * hl + 65 * qb:
                                       130 * hl + 65 * qb + 64],
                                rcp[:, 2 * hl + qb:2 * hl + qb + 1])

                fillers = list(fillers)
                stride = max(1, (len(steps) + len(fillers)) // (len(fillers) + 1)) \
                    if fillers else 0
                yt2_of = {}
                pend = None         # (step, p_b)
                for si, st in enumerate(steps):
                    g = st[0]
                    if g not in yt2_of:
                        # drain previous group before its bank is reused
                        if pend is not None and pend[0][0] != g:
                            emit_pv(pend[0], pend[1], yt2_of[pend[0][0]])
                            pend = None
                        if g - 1 in yt2_of:
                            drain(g - 1, yt2_of[g - 1])
                        yt2_of[g] = psY.tile([128, 390], f32, tag="yt2",
                                             name="yt2")
                    p_b = emit_s(st)
                    if pend is not None:
                        emit_pv(pend[0], pend[1], yt2_of[pend[0][0]])
                    pend = (st, p_b)
                    if fillers and si % stride == stride - 1:
                        fillers.pop(0)()
                if pend is not None:
                    emit_pv(pend[0], pend[1], yt2_of[pend[0][0]])
                drain(HKV - 1, yt2_of[HKV - 1])
                for f in fillers:
                    f()

            _ysb_cache = {}

            def _ysb_for(s, qb):
                key = (s, qb)
                if key not in _ysb_cache:
                    _ysb_cache[key] = ysp.tile([128, 576], bf16, tag="ysb",
                                               name=f"ysb{s}_{qb}")
                return _ysb_cache[key]

            def finish_qb(s, qb):
                """transpose ysb -> ypr for one query-128-block of slot s."""
                ysb = _ysb_cache.pop((s, qb))
                tpp = psT.tile([128, 640], bf16, tag="tpp", name="tpp")
                for p, (pc0, pl) in enumerate(MM):
                    nc.tensor.transpose(tpp[:pl, 128 * p:128 * (p + 1)],
                                        ysb[:, pc0:pc0 + pl],
                                        ident_t[:, :])
                for p, (pc0, pl) in enumerate(MM):
                    nc.vector.tensor_copy(
                        ypr[p][:pl, QB * s + 128 * qb:QB * s + 128 * qb + 128],
                        tpp[:pl, 128 * p:128 * (p + 1)])

            def attn_finish(s):
                for qb in range(2):
                    finish_qb(s, qb)

            def attn_slot(s, fillers=()):
                attn_groups(s, SLOT_GROUPS[s], fillers=fillers)
                attn_finish(s)

            # ---------------- output projection ------------------------
            def oproj_qi(qi, act_copy=False):
                    psr = psA.tile([128, 512], f32, tag="pja", name="pja")
                    for p, (pc0, pl) in enumerate(MM):
                        nc.tensor.matmul(psr[:, :],
                                         ypr[p][:pl, 128 * qi:128 * (qi + 1)],
                                         wo_r[p][:pl, 0:512],
                                         start=(p == 0), stop=(p == 4))
                    psr2 = psB.tile([128, 512], f32, tag="pjb", name="pjb")
                    for p, (pc0, pl) in enumerate(MM):
                        nc.tensor.matmul(psr2[:, :64],
                                         ypr[p][:pl, 128 * qi:128 * (qi + 1)],
                                         wo_r[p][:pl, 512:576],
                                         start=(p == 0), stop=(p == 4))
                    ost = ysp.tile([128, 576], f32, tag="ost", name="ost")
                    if act_copy:
                        nc.scalar.activation(ost[:, 0:512], psr[:], AF.Copy)
                    else:
                        nc.vector.tensor_copy(ost[:, 0:512], psr[:])
                    nc.vector.tensor_copy(ost[:, 512:576], psr2[:, :64])
                    dma_eng = nc.scalar if qi % 2 else nc.sync
                    dma_eng.dma_start(yT[128 * qi:128 * (qi + 1), :], ost[:])

            # ---------------- schedule ---------------------------------
            # KV windows run in order 1,2,0,3 so slots 3 and 2 (which need
            # chunks 4-11) unlock after two windows and the exp stream on
            # ACT starts early. Later projection windows and O-proj blocks
            # ride as fillers inside the exp-bound attention sections, and
            # slots 0 and 1 are split in two passes (SBUF spill) so their
            # exp work spreads across the whole kernel instead of the tail.
            attn = "attn" not in ablate
            opj = "oproj" not in ablate and attn
            for u in qproj_units(1, xw1):        # queries 512:1024
                u()
            for u in kvproj_units(1, xw1, act_copy=True):   # key chunks 4-7
                u()
            for u in kvproj_units(2, xw2, act_copy=True):   # key chunks 8-11
                u()
            if attn:
                attn_slot(3, fillers=qproj_units(0, xw0))   # chunks 6-9
                attn_slot(2, fillers=kvproj_units(0, xw0))  # chunks 4-11
                # slot 1 pass A: first two groups (chunks 4-11)
                attn_groups(1, SLOT_GROUPS[1][:2], spill=True, last=False,
                            fillers=[lambda: oproj_qi(6), lambda: oproj_qi(7)]
                            if opj else ())
            if attn:
                # slot 0 pass A: first two groups (chunks 2-9)
                attn_groups(0, SLOT_GROUPS[0][:2], spill=True, last=False,
                            fillers=kvproj_units(3, xw3))   # chunks 12-15
                # slot 1 pass B: last group (chunks 12,13 + diag 2,3)
                attn_groups(1, SLOT_GROUPS[1][2:], spill=True, first=False,
                            fillers=[lambda: oproj_qi(4), lambda: oproj_qi(5)]
                            if opj else ())
                attn_finish(1)
                # slot 0 pass B: last two groups (chunks 10-15, 0-1)
                attn_groups(0, SLOT_GROUPS[0][2:], spill=True, first=False,
                            fillers=[lambda: oproj_qi(2), lambda: oproj_qi(3)]
                            if opj else ())
                finish_qb(0, 0)
                if "oproj" not in ablate:
                    oproj_qi(0, act_copy=True)
                finish_qb(0, 1)
                if "oproj" not in ablate:
                    oproj_qi(1, act_copy=True)

    nc.compile()
    return nc


def _get_program():
    global _PROG
    if _PROG is None:
        _PROG = _build_program()
    return _PROG


def _neox_perm(nheads, swap=False):
    p = []
    for h in range(nheads):
        ev = [64 * h + 2 * j for j in range(32)]
        od = [64 * h + 2 * j + 1 for j in range(32)]
        p += (od + ev) if swap else (ev + od)
    return np.array(p)


_CONSTS = None


def _static_consts():
    """Input-independent per-core constants (tables, masks, key orders)."""
    global _CONSTS
    if _CONSTS is not None:
        return _CONSTS
    invf = THETA ** (-np.arange(32, dtype=np.float64) / 32)

    def tables(pos):
        ang = pos[None, :] * invf[:, None]
        cos, sin = np.cos(ang), np.sin(ang)
        c2 = np.tile(cos, (4, 1)).astype(np.float32)
        s2 = np.tile(np.vstack([-sin, sin]), (2, 1)).astype(np.float32)
        return c2, s2

    per_j = []
    for j in range(2):
        keypos = np.concatenate(
            [np.arange(QB * q, QB * (q + 1)) for q in KEYORDER[j]])
        qsel = keypos[:TQ]          # queries = first 1024 permuted keys
        c2k, s2k = tables(keypos.astype(np.float64))
        masks = np.zeros((N_MASK * 128, QB), np.float32)
        for s in range(4):
            grp = SLOT_GROUPS[s][-1]
            qpos = keypos[QB * s:QB * (s + 1)]
            for k, c in enumerate(grp):
                kpos = keypos[128 * c:128 * (c + 1)]
                mi = MASK_OFF[s] + k
                masks[mi * 128:(mi + 1) * 128] = (
                    kpos[:, None] <= qpos[None, :]).astype(np.float32)
        per_j.append((keypos, qsel,
                      c2k.astype(ml_dtypes.bfloat16),
                      s2k.astype(ml_dtypes.bfloat16),
                      masks.astype(ml_dtypes.bfloat16)))
    _CONSTS = per_j
    return _CONSTS


def _perm_matrix():
    """128x128 block-diag pair-swap (per 64 rows: swap 32-halves)."""
    p = np.zeros((128, 128), np.float32)
    for b in range(2):
        for i in range(32):
            p[64 * b + 32 + i, 64 * b + i] = 1.0        # out[m]=in[swap(m)]
            p[64 * b + i, 64 * b + 32 + i] = 1.0
    return p.astype(ml_dtypes.bfloat16)


def _host_prep(x, Wq, Wk, Wv, Wo):
    wqT = _rne12(Wq[_neox_perm(H)].T)
    wkT = _rne12(Wk[_neox_perm(HKV)].T)
    woT = Wo.T.astype(ml_dtypes.bfloat16)
    wvT = np.zeros((577, 260), np.float32)
    for g in range(HKV):
        wvT[:C, 65 * g:65 * g + 64] = Wv[64 * g:64 * g + 64].T
        wvT[576, 65 * g + 64] = 1.0
    wvT = _rne12(wvT)
    perm = _perm_matrix()
    ident = np.eye(128, dtype=np.float32).astype(ml_dtypes.bfloat16)

    per_j = _static_consts()
    x = _rne12(x)
    ones = np.ones((1, T), np.float32)
    in_maps = []
    core_meta = []
    for b in range(B):
        xbT = x[b].T
        for j in range(2):
            keypos, qsel, c2k, s2k, masks = per_j[j]
            xkT = np.vstack([xbT[:, keypos], ones])
            in_maps.append({
                "xkT": xkT,
                "wqT": wqT, "wkT": wkT, "wvT": wvT, "woT": woT,
                "c2k": c2k, "s2k": s2k,
                "masks": masks, "perm": perm, "ident": ident,
            })
            core_meta.append((b, qsel))
    return in_maps, core_meta


def kernel(x, Wq, Wk, Wv, Wo):
    x = np.asarray(x, np.float32)
    Wq = np.asarray(Wq, np.float32)
    Wk = np.asarray(Wk, np.float32)
    Wv = np.asarray(Wv, np.float32)
    Wo = np.asarray(Wo, np.float32)

    from concourse.bass_utils import run_bass_kernel_spmd

    nc = _get_program()
    in_maps, core_meta = _host_prep(x, Wq, Wk, Wv, Wo)
    res = run_bass_kernel_spmd(nc, in_maps, list(range(8)))

    out = np.empty((B, T, C), np.float32)
    for core, (b, qsel) in enumerate(core_meta):
        out[b, qsel, :] = res.results[core]["yT"]
    return out


# revision 62
# speedup vs baseline: 1.5234x; 1.0327x over previous
"""Trainium2 Bass kernel for CausalSelfAttention (RoPE + GQA), 8-core SPMD.

Sharding: 8 cores = 4 batches x 2 query-halves. Each core owns four
query-256-blocks paired {i, 7-i} so causal work is balanced. Keys are
PERMUTED per core: block order = [own q-blocks (desc causal depth), then
remaining blocks ascending]. Slot s's key chunks occupy the static range
starting at chunk 2s; its diagonal chunks are 2s..2s+1 (emitted last in
each slot so one bf16 mask multiply covers them), and the first 1024 key
columns ARE the core's queries -- Q-projection re-reads the same xkT
input tiles and the K RoPE tables double as Q tables. Every core runs an
identical instruction stream; all variation is input data.

Device pipeline per core (160us vs the 236us baseline):
  * RoPE pair-swap via a single 128x128 permutation matmul on the raw
    projection (PSUM -> bf16 SBUF copy -> perm matmul) instead of a
    second full 5-chunk projection: 6 instead of 10 matmuls per chunk.
  * P.V flipped: out[q, d] with queries on partitions (65-row moving V in
    bf16) instead of out[d+1, q] with 256-row moving P -- half the PE
    rows; softmax denominator comes per-partition so the divide is a
    cheap tensor_scalar; a bf16 PE transpose restores [d, q] for O-proj,
    whose out[q, m] form then streams bf16 woT as the moving operand.
  * One PSUM accumulation group per yt2 bank (start on first PV matmul,
    stop on last; interior regions zero on first touch).
  * exp on ACT is the co-critical ~93us floor next to PE's ~102us: the
    wavefront schedule (KV windows in order 1,2,0,3) starts the exp
    stream early, slots 0 and 1 are split in two passes (SBUF spill +
    merge) so their exp spreads forward, and later projection windows /
    O-proj blocks ride as fillers inside the exp-bound attention
    sections; PV matmuls are emitted one step behind the next S group so
    the in-order PE queue never waits on the current exp.
  * Weight/mask/x loads are batched multi-level-AP DMAs split across the
    SP and ACT HWDGE queues in consumer order (queue issue costs
    ~0.7us each); first window + wq stay per-chunk so matmul 0 starts
    at ~4.5us.
  * bf16 for everything off the f32r spine (P, V, masks, q/k rope
    outputs, rope tables, Wo, y) -- rel err ~6e-3 vs the 2e-2 gate.
"""
import sys

sys.path.insert(0, "/opt/trn_rl_repo")

import numpy as np
import ml_dtypes

B, T, C = 4, 2048, 576
H, HKV, D = 9, 3, 64
THETA = 10000.0
QB = 256                      # query block
TQ = 1024                     # queries per core
QBLOCKS = [[7, 5, 2, 0], [6, 4, 3, 1]]   # q-256-block ids per half j
KEYORDER = [[7, 5, 2, 0, 1, 3, 4, 6], [6, 4, 3, 1, 0, 2, 5, 7]]
CCX = [(0, 128), (128, 128), (256, 128), (384, 128), (512, 65)]   # x chunks (577 rows incl ones)
CCQ = [(0, 128), (128, 128), (256, 128), (384, 128), (512, 64)]   # 576-row chunks
MM = [(0, 128), (128, 128), (256, 128), (384, 128), (512, 64)]    # output-dim chunks of 576

# per-slot key-chunk groups (exp granularity); last group of each slot is
# the masked one (diagonal chunks 2s, 2s+1 emitted last). Slot s spans the
# static chunk range [2s, 2s + pad_s) with pad = [16, 12, 8, 4]: the
# per-slot max of the two query-half profiles ([16,12,6,2] for blocks
# [7,5,2,0] and [14,10,8,4] for [6,4,3,1]) -- the host-side mask data
# resolves which chunks are visible per core.
SLOT_GROUPS = [
    [(2, 3, 4, 5), (6, 7, 8, 9), (10, 11, 12, 13), (14, 15, 0, 1)],
    [(4, 5, 6, 7), (8, 9, 10, 11), (12, 13, 2, 3)],
    [(6, 7, 8, 9), (10, 11, 4, 5)],
    [(8, 9, 6, 7)],
]
MASK_W = [4, 4, 4, 4]                      # masked-group widths (chunks)
MASK_OFF = [0, 4, 8, 12]                   # mask-chunk offset per slot
N_MASK = 16

_PROG = None


def _rne12(x):
    """Round fp32 to f32r (RNE, drop 12 mantissa bits) -- matches TRN2."""
    b = np.ascontiguousarray(x, np.float32).view(np.uint32).astype(np.uint64)
    lsb = (b >> np.uint64(12)) & np.uint64(1)
    r = (b + np.uint64(2047) + lsb) >> np.uint64(12) << np.uint64(12)
    return (r & np.uint64(0xFFFFFFFF)).astype(np.uint32).view(np.float32)


def _build_program(ablate=()):
    import concourse.bacc as bacc
    import concourse.mybir as mybir
    import concourse.tile as tile

    dt = mybir.dt
    f32, f32r, bf16 = dt.float32, dt.float32r, dt.bfloat16
    AF = mybir.ActivationFunctionType

    nc = bacc.Bacc("TRN2", target_bir_lowering=False, debug=False, num_devices=8)

    def inp(name, shape, d=f32):
        return nc.declare_dram_parameter(name, shape, d, isOutput=False)

    xkT = inp("xkT", [577, T], f32r)
    wqT = inp("wqT", [C, C], f32r)
    wkT = inp("wkT", [C, HKV * D], f32r)
    wvT = inp("wvT", [577, 260], f32r)
    woT = inp("woT", [C, C], bf16)
    c2k = inp("c2k", [128, T], bf16)
    s2k = inp("s2k", [128, T], bf16)
    masksp = inp("masks", [N_MASK * 128, QB], bf16)
    permp = inp("perm", [128, 128], bf16)
    identp = inp("ident", [128, 128], bf16)
    yT = nc.declare_dram_parameter("yT", [TQ, C], f32, isOutput=True)

    with tile.TileContext(nc) as tc:
        with (
            tc.tile_pool(name="const", bufs=1) as cp,
            tc.tile_pool(name="xw", bufs=4) as xwp,          # x window tiles
            tc.tile_pool(name="qraw", bufs=3) as qrp,
            tc.tile_pool(name="rope", bufs=3) as rp,
            tc.tile_pool(name="pb", bufs=8) as pbp,
            tc.tile_pool(name="ysb", bufs=8) as ysp,
            tc.tile_pool(name="small", bufs=3) as smp,
            # PSUM: scores 2x2 banks + y accum 1 + transpose 1 = 6; the
            # projection pools (psA/psB or psR) use the remaining 2.
            tc.tile_pool(name="psS", bufs=2, space="PSUM") as psS,
            tc.tile_pool(name="psY", bufs=1, space="PSUM") as psY,
            tc.tile_pool(name="psT", bufs=1, space="PSUM") as psT,
            tc.tile_pool(name="psA", bufs=1, space="PSUM") as psA,
            tc.tile_pool(name="psB", bufs=1, space="PSUM") as psB,
        ):
            # ---------------- const loads ------------------------------
            # Two HWDGE queues: SP carries the x windows (critical path to
            # the first matmuls), the ACT queue carries weights/tables/masks
            # in consumer order, so startup is not serialized on one queue.
            def load_w(pool, param, chunks, cols, tag, d=f32r, eng=None):
                """Load a row-chunked weight as ONE tile via two batched
                DMAs (4 full 128-row chunks + the partial tail chunk) --
                each HWDGE queue issue costs ~0.7us of SEQ time, so DMA
                count matters more than transfer size here."""
                eng = eng or nc.sync
                nch = len(chunks)
                t = pool.tile([128, nch * cols], d, tag=tag, name=tag)
                nf = nch - 1
                eng.dma_start(
                    t[:, 0:nf * cols].rearrange("b (a c) -> b a c", a=nf),
                    param[0:128 * nf, :].rearrange("(a b) c -> b a c", a=nf))
                k0, kl = chunks[-1]
                eng.dma_start(t[:kl, nf * cols:], param[k0:k0 + kl, :])
                return [t[:, i * cols:(i + 1) * cols] for i in range(nch)]

            def load_xwin(w, eng=None):
                eng = eng or nc.sync
                t = xwp.tile([128, 5 * 512], f32r, tag="xk", name=f"xw{w}")
                eng.dma_start(
                    t[:, 0:4 * 512].rearrange("b (a c) -> b a c", a=4),
                    xkT[0:512, 512 * w:512 * (w + 1)]
                    .rearrange("(a b) c -> b a c", a=4))
                eng.dma_start(t[:65, 4 * 512:],
                              xkT[512:577, 512 * w:512 * (w + 1)])
                return [t[:, i * 512:(i + 1) * 512] for i in range(5)]

            # first window + wq load per-chunk so the first projection
            # matmuls start as soon as chunk 0 lands, not after the batch
            def load_xwin1():
                t = xwp.tile([128, 5 * 512], f32r, tag="xk", name="xw1")
                for i, (k0, kl) in enumerate(CCX):
                    nc.sync.dma_start(t[:kl, 512 * i:512 * (i + 1)],
                                      xkT[k0:k0 + kl, 512:1024])
                return [t[:, i * 512:(i + 1) * 512] for i in range(5)]

            def load_wq():
                t = cp.tile([128, 5 * C], f32r, tag="wq", name="wq")
                for i, (k0, kl) in enumerate(CCQ):
                    nc.scalar.dma_start(t[:kl, C * i:C * (i + 1)],
                                        wqT[k0:k0 + kl, :])
                return [t[:, i * C:(i + 1) * C] for i in range(5)]

            xw1 = load_xwin1()
            wq_r = load_wq()
            c2k_t = cp.tile([128, T], bf16, tag="c2k", name="c2k")
            s2k_t = cp.tile([128, T], bf16, tag="s2k", name="s2k")
            perm_t = cp.tile([128, 128], bf16, tag="perm", name="perm")
            ident_t = cp.tile([128, 128], bf16, tag="ident", name="ident")
            nc.scalar.dma_start(perm_t[:], permp[:])
            nc.scalar.dma_start(c2k_t[:], c2k[:])
            nc.scalar.dma_start(s2k_t[:], s2k[:])
            def load_xwin2():
                t = xwp.tile([128, 5 * 512], f32r, tag="xk", name="xw2")
                for i, (k0, kl) in enumerate(CCX):
                    nc.sync.dma_start(t[:kl, 512 * i:512 * (i + 1)],
                                      xkT[k0:k0 + kl, 1024:1536])
                return [t[:, i * 512:(i + 1) * 512] for i in range(5)]

            xw2 = load_xwin2()
            xw0 = load_xwin(0)
            xw3 = load_xwin(3)
            wk_r = load_w(cp, wkT, CCQ, HKV * D, "wk", eng=nc.scalar)
            wv_r = load_w(cp, wvT, CCX, 260, "wv", eng=nc.scalar)
            m_b = cp.tile([128, N_MASK * QB], bf16, tag="masks", name="masks")
            nc.scalar.dma_start(
                m_b[:, :].rearrange("b (a c) -> b a c", a=N_MASK),
                masksp[:, :].rearrange("(a b) c -> b a c", a=N_MASK))
            nc.scalar.dma_start(ident_t[:], identp[:])
            wo_r = load_w(cp, woT, MM, C, "wo", d=bf16, eng=nc.scalar)

            # persistent projection outputs
            kt_h = [cp.tile([64, T], bf16, tag=f"kt{g}", name=f"kt{g}")
                    for g in range(HKV)]
            qth = [cp.tile([64, TQ], bf16, tag=f"qth{h}", name=f"qth{h}")
                   for h in range(H)]
            v_t = [cp.tile([128, 260], bf16, tag=f"v{c}", name=f"v{c}")
                   for c in range(16)]
            ypr = [cp.tile([128, TQ], bf16, tag=f"ypr{p}", name=f"ypr{p}")
                   for p in range(5)]
            spill_t = {}

            def spill_tile(s_, g):
                if (s_, g) not in spill_t:
                    spill_t[(s_, g)] = cp.tile(
                        [128, 390], f32, tag=f"ysp{s_}_{g}", name=f"ysp{s_}_{g}")
                return spill_t[(s_, g)]

            # ---------------- rope: proj + perm-matmul + combine -------
            def rope_chunk(xr, w_r, mc0, mrows, cols0, dsts, kside=False,
                           act_copy=False):
                """project chunk -> rope -> dsts[bi][0:64, cols0:cols0+512].
                K-side SBUF-only work (t1 mul, adds) goes to the otherwise
                idle Pool engine; PSUM-reading ops must stay on DVE/ACT.
                act_copy routes the PSUM drain to ACT -- used in sections
                where no exp stream is running yet."""
                ps = psA.tile([128, 512], f32, tag="pja", name="pja")
                for ci, (k0, kl) in enumerate(CCQ):
                    nc.tensor.matmul(ps[:mrows, :],
                                     w_r[ci][:kl, mc0:mc0 + mrows],
                                     xr[ci][:kl, :],
                                     start=(ci == 0), stop=(ci == 4))
                qraw = qrp.tile([128, 512], bf16, tag="qraw", name="qraw")
                if act_copy:
                    nc.scalar.activation(qraw[:mrows, :], ps[:mrows, :], AF.Copy)
                else:
                    nc.vector.tensor_copy(qraw[:mrows, :], ps[:mrows, :])
                ps2 = psB.tile([128, 512], f32, tag="pjb", name="pjb")
                nc.tensor.matmul(ps2[:mrows, :], perm_t[:mrows, :mrows],
                                 qraw[:mrows, :], start=True, stop=True)
                t1 = rp.tile([128, 512], bf16, tag="rope1", name="rope1")
                t2 = rp.tile([128, 512], bf16, tag="rope2", name="rope2")
                t1eng = nc.gpsimd if kside else nc.vector
                t1eng.tensor_mul(t1[:mrows, :], qraw[:mrows, :],
                                 c2k_t[:mrows, cols0:cols0 + 512])
                nc.vector.tensor_mul(t2[:mrows, :], ps2[:mrows, :],
                                     s2k_t[:mrows, cols0:cols0 + 512])
                for bi, dt_ in enumerate(dsts):
                    eng = nc.gpsimd if (kside or bi % 2) else nc.vector
                    eng.tensor_add(dt_[0:64, cols0:cols0 + 512],
                                   t1[64 * bi:64 * bi + 64, :],
                                   t2[64 * bi:64 * bi + 64, :])

            def qproj_units(w, xr):
                units = []
                for m, (mc0, mrows) in enumerate(MM):
                    dsts = ([qth[2 * m], qth[2 * m + 1]] if m < 4 else [qth[8]])
                    units.append(lambda mc0=mc0, mrows=mrows, dsts=dsts:
                                 rope_chunk(xr, wq_r, mc0, mrows, 512 * w,
                                            dsts))
                return units

            def vproj_one(w, xr, ti, act_copy=False):
                t_ = 4 * w + ti
                vpool = psA if ti % 2 == 0 else psB
                vtag = "pja" if ti % 2 == 0 else "pjb"
                ps = vpool.tile([128, 512], f32, tag=vtag, name=vtag)
                for ci, (k0, kl) in enumerate(CCX):
                    nc.tensor.matmul(ps[:, :260],
                                     xr[ci][:kl, 128 * ti:128 * (ti + 1)],
                                     wv_r[ci][:kl, :],
                                     start=(ci == 0), stop=(ci == 4))
                if act_copy:
                    nc.scalar.activation(v_t[t_][:], ps[:, :260], AF.Copy)
                else:
                    nc.vector.tensor_copy(v_t[t_][:], ps[:, :260])

            def kvproj_units(w, xr, act_copy=False):
                units = []
                for mi, (mc0, mrows) in enumerate([(0, 128), (128, 64)]):
                    dsts = [kt_h[0], kt_h[1]] if mi == 0 else [kt_h[2]]
                    units.append(lambda mc0=mc0, mrows=mrows, dsts=dsts:
                                 rope_chunk(xr, wk_r, mc0, mrows, 512 * w,
                                            dsts, kside=True,
                                            act_copy=act_copy))
                for ti in range(4):
                    units.append(lambda ti=ti: vproj_one(w, xr, ti))
                return units

            def qproj_win(w, xr):
                for u in qproj_units(w, xr):
                    u()

            def kvproj_win(w, xr):
                for u in kvproj_units(w, xr):
                    u()

            # ---------------- attention --------------------------------
            def attn_groups(s, groups, spill=None, first=True, last=True,
                            accum=False, fillers=()):
                """Process score/PV groups for slot s. If spill is given and
                last=False, accumulate into psY then copy to the slot's spill
                tiles (partial pass A); if spill is given and first=False,
                merge the spill into the final result.

                Emission is software-pipelined: the PV matmuls for step i are
                emitted after the S matmuls of step i+1 so the in-order PE
                queue never waits on the exp (ACT) of the current step.
                `fillers` are independent PE work units (projection chunks,
                O-proj blocks) interleaved between steps to cover the
                exp-bound deficit of the attention pipeline."""
                nchunks = sum(len(gr) for gr in groups)
                steps = []          # (g, hl, sc, grp, ci0)
                for g in range(HKV):
                    for hl in range(3):
                        ci0 = 0
                        for sc, grp in enumerate(groups):
                            steps.append((g, hl, sc, grp, ci0))
                            ci0 += len(grp)

                def emit_s(st):
                    g, hl, sc, grp, ci0 = st
                    h = 3 * g + hl
                    sp = psS.tile([128, 1024], f32, tag="scores", name="scores")
                    for i, c_ in enumerate(grp):
                        nc.tensor.matmul(
                            sp[:, QB * i:QB * (i + 1)],
                            kt_h[g][0:64, 128 * c_:128 * (c_ + 1)],
                            qth[h][0:64, QB * s:QB * (s + 1)],
                            start=True, stop=True)
                    wgrp = len(grp)
                    p_b = pbp.tile([128, 1024], bf16, tag="p", name="p")
                    nc.scalar.activation(p_b[:, :QB * wgrp], sp[:, :QB * wgrp],
                                         AF.Exp, scale=0.125)
                    if last and sc == len(groups) - 1:
                        mo = QB * MASK_OFF[s]
                        nc.vector.tensor_mul(
                            p_b[:, :QB * wgrp], p_b[:, :QB * wgrp],
                            m_b[:, mo:mo + QB * wgrp])
                    return p_b

                def emit_pv(st, p_b, yt2):
                    # one accumulation group per PSUM bank: start on the very
                    # first matmul into yt2, stop on the very last; interior
                    # regions are zeroed on first touch (pending-zero).
                    g, hl, sc, grp, ci0 = st
                    for i, c_ in enumerate(grp):
                        for qb in range(2):
                            nc.tensor.matmul(
                                yt2[:, 130 * hl + 65 * qb:
                                    130 * hl + 65 * qb + 65],
                                p_b[:, QB * i + 128 * qb:
                                    QB * i + 128 * qb + 128],
                                v_t[c_][:, 65 * g:65 * g + 65],
                                start=(hl == 0 and ci0 + i == 0 and qb == 0),
                                stop=(hl == 2 and ci0 + i == nchunks - 1
                                      and qb == 1))

                def drain(g, yt2):
                    if spill is not None and not last:
                        if accum:
                            nc.vector.tensor_add(spill_tile(s, g)[:], yt2[:],
                                                 spill_tile(s, g)[:])
                        else:
                            nc.vector.tensor_copy(spill_tile(s, g)[:], yt2[:])
                        return
                    merged = yt2
                    merged_sbuf = False
                    if spill is not None and not first:
                        msb = smp.tile([128, 390], f32, tag="merged",
                                       name="merged")
                        nc.vector.tensor_add(msb[:], yt2[:], spill_tile(s, g)[:])
                        merged = msb
                        merged_sbuf = True
                    rcp = smp.tile([128, 6], f32, tag="rcp", name="rcp")
                    nc.vector.reciprocal(rcp[:], merged[:, 64::65])
                    # SBUF-resident merge (slot-0 pass B) can divide on Pool
                    ts_eng = nc.gpsimd if merged_sbuf else nc.vector
                    for hl in range(3):
                        h = 3 * g + hl
                        for qb in range(2):
                            ysb = _ysb_for(s, qb)
                            ts_eng.tensor_scalar_mul(
                                ysb[:, 64 * h:64 * h + 64],
                                merged[:, 130 * hl + 65 * qb:
                                       130 * hl + 65 * qb + 64],
                                rcp[:, 2 * hl + qb:2 * hl + qb + 1])

                fillers = list(fillers)
                stride = max(1, (len(steps) + len(fillers)) // (len(fillers) + 1)) \
                    if fillers else 0
                yt2_of = {}
                pend = None         # (step, p_b)
                for si, st in enumerate(steps):
                    g = st[0]
                    if g not in yt2_of:
                        # drain previous group before its bank is reused
                        if pend is not None and pend[0][0] != g:
                            emit_pv(pend[0], pend[1], yt2_of[pend[0][0]])
                            pend = None
                        if g - 1 in yt2_of:
                            drain(g - 1, yt2_of[g - 1])
                        yt2_of[g] = psY.tile([128, 390], f32, tag="yt2",
                                             name="yt2")
                    p_b = emit_s(st)
                    if pend is not None:
                        emit_pv(pend[0], pend[1], yt2_of[pend[0][0]])
                    pend = (st, p_b)
                    if fillers and si % stride == stride - 1:
                        fillers.pop(0)()
                if pend is not None:
                    emit_pv(pend[0], pend[1], yt2_of[pend[0][0]])
                drain(HKV - 1, yt2_of[HKV - 1])
                for f in fillers:
                    f()

            _ysb_cache = {}

            def _ysb_for(s, qb):
                key = (s, qb)
                if key not in _ysb_cache:
                    _ysb_cache[key] = ysp.tile([128, 576], bf16, tag="ysb",
                                               name=f"ysb{s}_{qb}")
                return _ysb_cache[key]

            def finish_qb(s, qb):
                """transpose ysb -> ypr for one query-128-block of slot s."""
                ysb = _ysb_cache.pop((s, qb))
                tpp = psT.tile([128, 640], bf16, tag="tpp", name="tpp")
                for p, (pc0, pl) in enumerate(MM):
                    nc.tensor.transpose(tpp[:pl, 128 * p:128 * (p + 1)],
                                        ysb[:, pc0:pc0 + pl],
                                        ident_t[:, :])
                for p, (pc0, pl) in enumerate(MM):
                    nc.vector.tensor_copy(
                        ypr[p][:pl, QB * s + 128 * qb:QB * s + 128 * qb + 128],
                        tpp[:pl, 128 * p:128 * (p + 1)])

            def attn_finish(s):
                for qb in range(2):
                    finish_qb(s, qb)

            def attn_slot(s, fillers=()):
                attn_groups(s, SLOT_GROUPS[s], fillers=fillers)
                attn_finish(s)

            # ---------------- output projection ------------------------
            def oproj_qi(qi, act_copy=False):
                    psr = psA.tile([128, 512], f32, tag="pja", name="pja")
                    for p, (pc0, pl) in enumerate(MM):
                        nc.tensor.matmul(psr[:, :],
                                         ypr[p][:pl, 128 * qi:128 * (qi + 1)],
                                         wo_r[p][:pl, 0:512],
                                         start=(p == 0), stop=(p == 4))
                    psr2 = psB.tile([128, 512], f32, tag="pjb", name="pjb")
                    for p, (pc0, pl) in enumerate(MM):
                        nc.tensor.matmul(psr2[:, :64],
                                         ypr[p][:pl, 128 * qi:128 * (qi + 1)],
                                         wo_r[p][:pl, 512:576],
                                         start=(p == 0), stop=(p == 4))
                    ost = ysp.tile([128, 576], f32, tag="ost", name="ost")
                    if act_copy:
                        nc.scalar.activation(ost[:, 0:512], psr[:], AF.Copy)
                    else:
                        nc.vector.tensor_copy(ost[:, 0:512], psr[:])
                    nc.vector.tensor_copy(ost[:, 512:576], psr2[:, :64])
                    dma_eng = nc.scalar if qi % 2 else nc.sync
                    dma_eng.dma_start(yT[128 * qi:128 * (qi + 1), :], ost[:])

            # ---------------- schedule ---------------------------------
            # KV windows run in order 1,2,0,3 so slots 3 and 2 (which need
            # chunks 4-11) unlock after two windows and the exp stream on
            # ACT starts early. Later projection windows and O-proj blocks
            # ride as fillers inside the exp-bound attention sections, and
            # slots 0 and 1 are split in two passes (SBUF spill) so their
            # exp work spreads across the whole kernel instead of the tail.
            attn = "attn" not in ablate
            opj = "oproj" not in ablate and attn
            for u in qproj_units(1, xw1):        # queries 512:1024
                u()
            for u in kvproj_units(1, xw1, act_copy=True):   # key chunks 4-7
                u()
            for u in kvproj_units(2, xw2, act_copy=True):   # key chunks 8-11
                u()
            if attn:
                attn_slot(3, fillers=qproj_units(0, xw0))   # chunks 6-9
                attn_slot(2, fillers=kvproj_units(0, xw0))  # chunks 4-11
                # slot 1 pass A: first two groups (chunks 4-11)
                attn_groups(1, SLOT_GROUPS[1][:2], spill=True, last=False,
                            fillers=[lambda: oproj_qi(6), lambda: oproj_qi(7)]
                            if opj else ())
            if attn:
                # slot 0 pass A: first two groups (chunks 2-9)
                attn_groups(0, SLOT_GROUPS[0][:2], spill=True, last=False,
                            fillers=kvproj_units(3, xw3))   # chunks 12-15
                # slot 1 pass B: last group (chunks 12,13 + diag 2,3)
                attn_groups(1, SLOT_GROUPS[1][2:], spill=True, first=False,
                            fillers=[lambda: oproj_qi(4), lambda: oproj_qi(5)]
                            if opj else ())
                attn_finish(1)
                # slot 0 pass B: last two groups (chunks 10-15, 0-1)
                attn_groups(0, SLOT_GROUPS[0][2:], spill=True, first=False,
                            fillers=[lambda: oproj_qi(2), lambda: oproj_qi(3)]
                            if opj else ())
                finish_qb(0, 0)
                if "oproj" not in ablate:
                    oproj_qi(0, act_copy=True)
                finish_qb(0, 1)
                if "oproj" not in ablate:
                    oproj_qi(1, act_copy=True)

    nc.compile()
    return nc


def _get_program():
    global _PROG
    if _PROG is None:
        _PROG = _build_program()
    return _PROG


def _neox_perm(nheads, swap=False):
    p = []
    for h in range(nheads):
        ev = [64 * h + 2 * j for j in range(32)]
        od = [64 * h + 2 * j + 1 for j in range(32)]
        p += (od + ev) if swap else (ev + od)
    return np.array(p)


_CONSTS = None


def _static_consts():
    """Input-independent per-core constants (tables, masks, key orders)."""
    global _CONSTS
    if _CONSTS is not None:
        return _CONSTS
    invf = THETA ** (-np.arange(32, dtype=np.float64) / 32)

    def tables(pos):
        ang = pos[None, :] * invf[:, None]
        cos, sin = np.cos(ang), np.sin(ang)
        c2 = np.tile(cos, (4, 1)).astype(np.float32)
        s2 = np.tile(np.vstack([-sin, sin]), (2, 1)).astype(np.float32)
        return c2, s2

    per_j = []
    for j in range(2):
        keypos = np.concatenate(
            [np.arange(QB * q, QB * (q + 1)) for q in KEYORDER[j]])
        qsel = keypos[:TQ]          # queries = first 1024 permuted keys
        c2k, s2k = tables(keypos.astype(np.float64))
        masks = np.zeros((N_MASK * 128, QB), np.float32)
        for s in range(4):
            grp = SLOT_GROUPS[s][-1]
            qpos = keypos[QB * s:QB * (s + 1)]
            for k, c in enumerate(grp):
                kpos = keypos[128 * c:128 * (c + 1)]
                mi = MASK_OFF[s] + k
                masks[mi * 128:(mi + 1) * 128] = (
                    kpos[:, None] <= qpos[None, :]).astype(np.float32)
        per_j.append((keypos, qsel,
                      c2k.astype(ml_dtypes.bfloat16),
                      s2k.astype(ml_dtypes.bfloat16),
                      masks.astype(ml_dtypes.bfloat16)))
    _CONSTS = per_j
    return _CONSTS


def _perm_matrix():
    """128x128 block-diag pair-swap (per 64 rows: swap 32-halves)."""
    p = np.zeros((128, 128), np.float32)
    for b in range(2):
        for i in range(32):
            p[64 * b + 32 + i, 64 * b + i] = 1.0        # out[m]=in[swap(m)]
            p[64 * b + i, 64 * b + 32 + i] = 1.0
    return p.astype(ml_dtypes.bfloat16)


def _host_prep(x, Wq, Wk, Wv, Wo):
    wqT = _rne12(Wq[_neox_perm(H)].T)
    wkT = _rne12(Wk[_neox_perm(HKV)].T)
    woT = Wo.T.astype(ml_dtypes.bfloat16)
    wvT = np.zeros((577, 260), np.float32)
    for g in range(HKV):
        wvT[:C, 65 * g:65 * g + 64] = Wv[64 * g:64 * g + 64].T
        wvT[576, 65 * g + 64] = 1.0
    wvT = _rne12(wvT)
    perm = _perm_matrix()
    ident = np.eye(128, dtype=np.float32).astype(ml_dtypes.bfloat16)

    per_j = _static_consts()
    x = _rne12(x)
    ones = np.ones((1, T), np.float32)
    in_maps = []
    core_meta = []
    for b in range(B):
        xbT = x[b].T
        for j in range(2):
            keypos, qsel, c2k, s2k, masks = per_j[j]
            xkT = np.vstack([xbT[:, keypos], ones])
            in_maps.append({
                "xkT": xkT,
                "wqT": wqT, "wkT": wkT, "wvT": wvT, "woT": woT,
                "c2k": c2k, "s2k": s2k,
                "masks": masks, "perm": perm, "ident": ident,
            })
            core_meta.append((b, qsel))
    return in_maps, core_meta


def kernel(x, Wq, Wk, Wv, Wo):
    x = np.asarray(x, np.float32)
    Wq = np.asarray(Wq, np.float32)
    Wk = np.asarray(Wk, np.float32)
    Wv = np.asarray(Wv, np.float32)
    Wo = np.asarray(Wo, np.float32)

    from concourse.bass_utils import run_bass_kernel_spmd

    nc = _get_program()
    in_maps, core_meta = _host_prep(x, Wq, Wk, Wv, Wo)
    res = run_bass_kernel_spmd(nc, in_maps, list(range(8)))

    out = np.empty((B, T, C), np.float32)
    for core, (b, qsel) in enumerate(core_meta):
        out[b, qsel, :] = res.results[core]["yT"]
    return out


# revision 74
# speedup vs baseline: 1.5323x; 1.0058x over previous
"""Trainium2 Bass kernel for CausalSelfAttention (RoPE + GQA), 8-core SPMD.

Sharding: 8 cores = 4 batches x 2 query-halves. Each core owns four
query-256-blocks paired {i, 7-i} so causal work is balanced. Keys are
PERMUTED per core: block order = [own q-blocks (desc causal depth), then
remaining blocks ascending]. Slot s's key chunks occupy the static range
starting at chunk 2s; its diagonal chunks are 2s..2s+1 (emitted last in
each slot so one bf16 mask multiply covers them), and the first 1024 key
columns ARE the core's queries -- Q-projection re-reads the same xkT
input tiles and the K RoPE tables double as Q tables. Every core runs an
identical instruction stream; all variation is input data.

Device pipeline per core (155us vs the 236us baseline):
  * RoPE pair-swap via a single 128x128 permutation matmul on the raw
    projection (PSUM -> bf16 SBUF copy -> perm matmul) instead of a
    second full 5-chunk projection: 6 instead of 10 matmuls per chunk.
  * P.V flipped: out[q, d] with queries on partitions (65-row moving V in
    bf16) instead of out[d+1, q] with 256-row moving P -- half the PE
    rows; softmax denominator comes per-partition so the divide is a
    cheap tensor_scalar; a bf16 PE transpose restores [d, q] for O-proj,
    whose out[q, m] form then streams bf16 woT as the moving operand.
  * One PSUM accumulation group per yt2 bank (start on first PV matmul,
    stop on last; interior regions zero on first touch).
  * exp on ACT is the co-critical ~93us floor next to PE's ~102us: the
    wavefront schedule (KV windows in order 1,2,0,3) starts the exp
    stream early, slots 0 and 1 are split in two passes (SBUF spill +
    merge) so their exp spreads forward, and later projection windows /
    O-proj blocks ride as fillers inside the exp-bound attention
    sections; PV matmuls are emitted one step behind the next S group so
    the in-order PE queue never waits on the current exp.
  * Weight/mask/x loads are batched multi-level-AP DMAs split across the
    SP and ACT HWDGE queues in consumer order (queue issue costs
    ~0.7us each); first window + wq stay per-chunk so matmul 0 starts
    at ~4.5us.
  * bf16 for everything off the f32r spine (P, V, masks, q/k rope
    outputs, rope tables, Wo, y) -- rel err ~6e-3 vs the 2e-2 gate.
"""
import sys

sys.path.insert(0, "/opt/trn_rl_repo")

import numpy as np
import ml_dtypes

B, T, C = 4, 2048, 576
H, HKV, D = 9, 3, 64
THETA = 10000.0
QB = 256                      # query block
TQ = 1024                     # queries per core
QBLOCKS = [[7, 5, 2, 0], [6, 4, 3, 1]]   # q-256-block ids per half j
KEYORDER = [[7, 5, 2, 0, 1, 3, 4, 6], [6, 4, 3, 1, 0, 2, 5, 7]]
CCX = [(0, 128), (128, 128), (256, 128), (384, 128), (512, 65)]   # x chunks (577 rows incl ones)
CCQ = [(0, 128), (128, 128), (256, 128), (384, 128), (512, 64)]   # 576-row chunks
MM = [(0, 128), (128, 128), (256, 128), (384, 128), (512, 64)]    # output-dim chunks of 576

# per-slot key-chunk groups (exp granularity); last group of each slot is
# the masked one (diagonal chunks 2s, 2s+1 emitted last). Slot s spans the
# static chunk range [2s, 2s + pad_s) with pad = [16, 12, 8, 4]: the
# per-slot max of the two query-half profiles ([16,12,6,2] for blocks
# [7,5,2,0] and [14,10,8,4] for [6,4,3,1]) -- the host-side mask data
# resolves which chunks are visible per core.
SLOT_GROUPS = [
    [(2, 3, 4, 5), (6, 7, 8, 9), (10, 11, 12, 13), (14, 15, 0, 1)],
    [(4, 5, 6, 7), (8, 9, 10, 11), (12, 13, 2, 3)],
    [(6, 7, 8, 9), (10, 11, 4, 5)],
    [(8, 9, 6, 7)],
]
MASK_W = [4, 4, 4, 4]                      # masked-group widths (chunks)
MASK_OFF = [0, 4, 8, 12]                   # mask-chunk offset per slot
N_MASK = 16

_PROG = None


def _rne12(x):
    """Round fp32 to f32r (RNE, drop 12 mantissa bits) -- matches TRN2."""
    b = np.ascontiguousarray(x, np.float32).view(np.uint32).astype(np.uint64)
    lsb = (b >> np.uint64(12)) & np.uint64(1)
    r = (b + np.uint64(2047) + lsb) >> np.uint64(12) << np.uint64(12)
    return (r & np.uint64(0xFFFFFFFF)).astype(np.uint32).view(np.float32)


def _build_program(ablate=()):
    import concourse.bacc as bacc
    import concourse.mybir as mybir
    import concourse.tile as tile

    dt = mybir.dt
    f32, f32r, bf16 = dt.float32, dt.float32r, dt.bfloat16
    AF = mybir.ActivationFunctionType

    nc = bacc.Bacc("TRN2", target_bir_lowering=False, debug=False, num_devices=8)

    def inp(name, shape, d=f32):
        return nc.declare_dram_parameter(name, shape, d, isOutput=False)

    xkT = inp("xkT", [577, T], f32r)
    wqT = inp("wqT", [C, C], f32r)
    wkT = inp("wkT", [C, HKV * D], f32r)
    wvT = inp("wvT", [577, 260], f32r)
    woT = inp("woT", [C, C], bf16)
    c2k = inp("c2k", [128, T], bf16)
    s2k = inp("s2k", [128, T], bf16)
    masksp = inp("masks", [N_MASK * 128, QB], bf16)
    permp = inp("perm", [128, 128], bf16)
    identp = inp("ident", [128, 128], bf16)
    yT = nc.declare_dram_parameter("yT", [TQ, C], f32, isOutput=True)

    with tile.TileContext(nc) as tc:
        with (
            tc.tile_pool(name="const", bufs=1) as cp,
            tc.tile_pool(name="xw", bufs=4) as xwp,          # x window tiles
            tc.tile_pool(name="qraw", bufs=3) as qrp,
            tc.tile_pool(name="rope", bufs=3) as rp,
            tc.tile_pool(name="pb", bufs=8) as pbp,
            tc.tile_pool(name="ysb", bufs=8) as ysp,
            tc.tile_pool(name="small", bufs=3) as smp,
            # PSUM: scores 2x2 banks + y accum 1 + transpose 1 = 6; the
            # projection pools (psA/psB or psR) use the remaining 2.
            tc.tile_pool(name="psS", bufs=2, space="PSUM") as psS,
            tc.tile_pool(name="psY", bufs=2, space="PSUM") as psY,
            tc.tile_pool(name="psA", bufs=1, space="PSUM") as psA,
            tc.tile_pool(name="psB", bufs=1, space="PSUM") as psB,
        ):
            # ---------------- const loads ------------------------------
            # Two HWDGE queues: SP carries the x windows (critical path to
            # the first matmuls), the ACT queue carries weights/tables/masks
            # in consumer order, so startup is not serialized on one queue.
            def load_w(pool, param, chunks, cols, tag, d=f32r, eng=None):
                """Load a row-chunked weight as ONE tile via two batched
                DMAs (4 full 128-row chunks + the partial tail chunk) --
                each HWDGE queue issue costs ~0.7us of SEQ time, so DMA
                count matters more than transfer size here."""
                eng = eng or nc.sync
                nch = len(chunks)
                t = pool.tile([128, nch * cols], d, tag=tag, name=tag)
                nf = nch - 1
                eng.dma_start(
                    t[:, 0:nf * cols].rearrange("b (a c) -> b a c", a=nf),
                    param[0:128 * nf, :].rearrange("(a b) c -> b a c", a=nf))
                k0, kl = chunks[-1]
                eng.dma_start(t[:kl, nf * cols:], param[k0:k0 + kl, :])
                return [t[:, i * cols:(i + 1) * cols] for i in range(nch)]

            def load_xwin(w, eng=None):
                eng = eng or nc.sync
                t = xwp.tile([128, 5 * 512], f32r, tag="xk", name=f"xw{w}")
                eng.dma_start(
                    t[:, 0:4 * 512].rearrange("b (a c) -> b a c", a=4),
                    xkT[0:512, 512 * w:512 * (w + 1)]
                    .rearrange("(a b) c -> b a c", a=4))
                eng.dma_start(t[:65, 4 * 512:],
                              xkT[512:577, 512 * w:512 * (w + 1)])
                return [t[:, i * 512:(i + 1) * 512] for i in range(5)]

            # first window + wq load per-chunk so the first projection
            # matmuls start as soon as chunk 0 lands, not after the batch
            def load_xwin1():
                t = xwp.tile([128, 5 * 512], f32r, tag="xk", name="xw1")
                for i, (k0, kl) in enumerate(CCX):
                    nc.sync.dma_start(t[:kl, 512 * i:512 * (i + 1)],
                                      xkT[k0:k0 + kl, 512:1024])
                return [t[:, i * 512:(i + 1) * 512] for i in range(5)]

            def load_wq():
                t = cp.tile([128, 5 * C], f32r, tag="wq", name="wq")
                for i, (k0, kl) in enumerate(CCQ):
                    nc.scalar.dma_start(t[:kl, C * i:C * (i + 1)],
                                        wqT[k0:k0 + kl, :])
                return [t[:, i * C:(i + 1) * C] for i in range(5)]

            xw1 = load_xwin1()
            wq_r = load_wq()
            c2k_t = cp.tile([128, T], bf16, tag="c2k", name="c2k")
            s2k_t = cp.tile([128, T], bf16, tag="s2k", name="s2k")
            perm_t = cp.tile([128, 128], bf16, tag="perm", name="perm")
            ident_t = cp.tile([128, 128], bf16, tag="ident", name="ident")
            nc.scalar.dma_start(perm_t[:], permp[:])
            nc.scalar.dma_start(c2k_t[:], c2k[:])
            nc.scalar.dma_start(s2k_t[:], s2k[:])
            def load_xwin2():
                t = xwp.tile([128, 5 * 512], f32r, tag="xk", name="xw2")
                for i, (k0, kl) in enumerate(CCX):
                    nc.sync.dma_start(t[:kl, 512 * i:512 * (i + 1)],
                                      xkT[k0:k0 + kl, 1024:1536])
                return [t[:, i * 512:(i + 1) * 512] for i in range(5)]

            xw2 = load_xwin2()
            xw0 = load_xwin(0)
            xw3 = load_xwin(3)
            wk_r = load_w(cp, wkT, CCQ, HKV * D, "wk", eng=nc.scalar)
            wv_r = load_w(cp, wvT, CCX, 260, "wv", eng=nc.scalar)
            m_b = cp.tile([128, N_MASK * QB], bf16, tag="masks", name="masks")
            nc.scalar.dma_start(
                m_b[:, :].rearrange("b (a c) -> b a c", a=N_MASK),
                masksp[:, :].rearrange("(a b) c -> b a c", a=N_MASK))
            nc.scalar.dma_start(ident_t[:], identp[:])
            wo_r = load_w(cp, woT, MM, C, "wo", d=bf16, eng=nc.scalar)

            # persistent projection outputs
            kt_h = [cp.tile([64, T], bf16, tag=f"kt{g}", name=f"kt{g}")
                    for g in range(HKV)]
            qth = [cp.tile([64, TQ], bf16, tag=f"qth{h}", name=f"qth{h}")
                   for h in range(H)]
            v_t = [cp.tile([128, 260], bf16, tag=f"v{c}", name=f"v{c}")
                   for c in range(16)]
            ypr = [cp.tile([128, TQ], bf16, tag=f"ypr{p}", name=f"ypr{p}")
                   for p in range(5)]
            spill_t = {}

            def spill_tile(s_, g):
                if (s_, g) not in spill_t:
                    spill_t[(s_, g)] = cp.tile(
                        [128, 390], f32, tag=f"ysp{s_}_{g}", name=f"ysp{s_}_{g}")
                return spill_t[(s_, g)]

            # ---------------- rope: proj + perm-matmul + combine -------
            def rope_chunk(xr, w_r, mc0, mrows, cols0, dsts, kside=False,
                           act_copy=False):
                """project chunk -> rope -> dsts[bi][0:64, cols0:cols0+512].
                K-side SBUF-only work (t1 mul, adds) goes to the otherwise
                idle Pool engine; PSUM-reading ops must stay on DVE/ACT.
                act_copy routes the PSUM drain to ACT -- used in sections
                where no exp stream is running yet."""
                ps = psA.tile([128, 512], f32, tag="pja", name="pja")
                for ci, (k0, kl) in enumerate(CCQ):
                    nc.tensor.matmul(ps[:mrows, :],
                                     w_r[ci][:kl, mc0:mc0 + mrows],
                                     xr[ci][:kl, :],
                                     start=(ci == 0), stop=(ci == 4))
                qraw = qrp.tile([128, 512], bf16, tag="qraw", name="qraw")
                if act_copy:
                    nc.scalar.activation(qraw[:mrows, :], ps[:mrows, :], AF.Copy)
                else:
                    nc.vector.tensor_copy(qraw[:mrows, :], ps[:mrows, :])
                ps2 = psB.tile([128, 512], f32, tag="pjb", name="pjb")
                nc.tensor.matmul(ps2[:mrows, :], perm_t[:mrows, :mrows],
                                 qraw[:mrows, :], start=True, stop=True)
                t1 = rp.tile([128, 512], bf16, tag="rope1", name="rope1")
                t2 = rp.tile([128, 512], bf16, tag="rope2", name="rope2")
                t1eng = nc.gpsimd if kside else nc.vector
                t1eng.tensor_mul(t1[:mrows, :], qraw[:mrows, :],
                                 c2k_t[:mrows, cols0:cols0 + 512])
                nc.vector.tensor_mul(t2[:mrows, :], ps2[:mrows, :],
                                     s2k_t[:mrows, cols0:cols0 + 512])
                for bi, dt_ in enumerate(dsts):
                    eng = nc.gpsimd if (kside or bi % 2) else nc.vector
                    eng.tensor_add(dt_[0:64, cols0:cols0 + 512],
                                   t1[64 * bi:64 * bi + 64, :],
                                   t2[64 * bi:64 * bi + 64, :])

            def qproj_units(w, xr):
                units = []
                for m, (mc0, mrows) in enumerate(MM):
                    dsts = ([qth[2 * m], qth[2 * m + 1]] if m < 4 else [qth[8]])
                    units.append(lambda mc0=mc0, mrows=mrows, dsts=dsts:
                                 rope_chunk(xr, wq_r, mc0, mrows, 512 * w,
                                            dsts))
                return units

            def vproj_one(w, xr, ti, act_copy=False):
                t_ = 4 * w + ti
                vpool = psA if ti % 2 == 0 else psB
                vtag = "pja" if ti % 2 == 0 else "pjb"
                ps = vpool.tile([128, 512], f32, tag=vtag, name=vtag)
                for ci, (k0, kl) in enumerate(CCX):
                    nc.tensor.matmul(ps[:, :260],
                                     xr[ci][:kl, 128 * ti:128 * (ti + 1)],
                                     wv_r[ci][:kl, :],
                                     start=(ci == 0), stop=(ci == 4))
                if act_copy:
                    nc.scalar.activation(v_t[t_][:], ps[:, :260], AF.Copy)
                else:
                    nc.vector.tensor_copy(v_t[t_][:], ps[:, :260])

            def kvproj_units(w, xr, act_copy=False):
                units = []
                for mi, (mc0, mrows) in enumerate([(0, 128), (128, 64)]):
                    dsts = [kt_h[0], kt_h[1]] if mi == 0 else [kt_h[2]]
                    units.append(lambda mc0=mc0, mrows=mrows, dsts=dsts:
                                 rope_chunk(xr, wk_r, mc0, mrows, 512 * w,
                                            dsts, kside=True,
                                            act_copy=act_copy))
                for ti in range(4):
                    units.append(lambda ti=ti: vproj_one(w, xr, ti))
                return units

            def qproj_win(w, xr):
                for u in qproj_units(w, xr):
                    u()

            def kvproj_win(w, xr):
                for u in kvproj_units(w, xr):
                    u()

            # ---------------- attention --------------------------------
            def attn_groups(s, groups, spill=None, first=True, last=True,
                            accum=False, fillers=()):
                """Process score/PV groups for slot s. If spill is given and
                last=False, accumulate into psY then copy to the slot's spill
                tiles (partial pass A); if spill is given and first=False,
                merge the spill into the final result.

                Emission is software-pipelined: the PV matmuls for step i are
                emitted after the S matmuls of step i+1 so the in-order PE
                queue never waits on the exp (ACT) of the current step.
                `fillers` are independent PE work units (projection chunks,
                O-proj blocks) interleaved between steps to cover the
                exp-bound deficit of the attention pipeline."""
                nchunks = sum(len(gr) for gr in groups)
                steps = []          # (g, hl, sc, grp, ci0)
                for g in range(HKV):
                    for hl in range(3):
                        ci0 = 0
                        for sc, grp in enumerate(groups):
                            steps.append((g, hl, sc, grp, ci0))
                            ci0 += len(grp)

                def emit_s(st):
                    g, hl, sc, grp, ci0 = st
                    h = 3 * g + hl
                    sp = psS.tile([128, 1024], f32, tag="scores", name="scores")
                    for i, c_ in enumerate(grp):
                        nc.tensor.matmul(
                            sp[:, QB * i:QB * (i + 1)],
                            kt_h[g][0:64, 128 * c_:128 * (c_ + 1)],
                            qth[h][0:64, QB * s:QB * (s + 1)],
                            start=True, stop=True)
                    wgrp = len(grp)
                    p_b = pbp.tile([128, 1024], bf16, tag="p", name="p")
                    nc.scalar.activation(p_b[:, :QB * wgrp], sp[:, :QB * wgrp],
                                         AF.Exp, scale=0.125)
                    if last and sc == len(groups) - 1:
                        mo = QB * MASK_OFF[s]
                        nc.vector.tensor_mul(
                            p_b[:, :QB * wgrp], p_b[:, :QB * wgrp],
                            m_b[:, mo:mo + QB * wgrp])
                    return p_b

                def emit_pv(st, p_b, yt2):
                    # one accumulation group per PSUM bank: start on the very
                    # first matmul into yt2, stop on the very last; interior
                    # regions are zeroed on first touch (pending-zero).
                    g, hl, sc, grp, ci0 = st
                    for i, c_ in enumerate(grp):
                        for qb in range(2):
                            nc.tensor.matmul(
                                yt2[:, 130 * hl + 65 * qb:
                                    130 * hl + 65 * qb + 65],
                                p_b[:, QB * i + 128 * qb:
                                    QB * i + 128 * qb + 128],
                                v_t[c_][:, 65 * g:65 * g + 65],
                                start=(hl == 0 and ci0 + i == 0 and qb == 0),
                                stop=(hl == 2 and ci0 + i == nchunks - 1
                                      and qb == 1))

                def drain(g, yt2):
                    if spill is not None and not last:
                        if accum:
                            nc.vector.tensor_add(spill_tile(s, g)[:], yt2[:],
                                                 spill_tile(s, g)[:])
                        else:
                            nc.vector.tensor_copy(spill_tile(s, g)[:], yt2[:])
                        return
                    merged = yt2
                    merged_sbuf = False
                    if spill is not None and not first:
                        msb = smp.tile([128, 390], f32, tag="merged",
                                       name="merged")
                        nc.vector.tensor_add(msb[:], yt2[:], spill_tile(s, g)[:])
                        merged = msb
                        merged_sbuf = True
                    rcp = smp.tile([128, 6], f32, tag="rcp", name="rcp")
                    nc.vector.reciprocal(rcp[:], merged[:, 64::65])
                    # keep the whole drain chain on DVE: cross-engine
                    # ping-pong adds ~100ns sem latency per hop
                    ts_eng = nc.vector
                    for hl in range(3):
                        h = 3 * g + hl
                        for qb in range(2):
                            ysb = _ysb_for(s, qb)
                            ts_eng.tensor_scalar_mul(
                                ysb[:, 64 * h:64 * h + 64],
                                merged[:, 130 * hl + 65 * qb:
                                       130 * hl + 65 * qb + 64],
                                rcp[:, 2 * hl + qb:2 * hl + qb + 1])

                fillers = list(fillers)
                stride = max(1, (len(steps) + len(fillers)) // (len(fillers) + 1)) \
                    if fillers else 0
                yt2_of = {}
                pend = None         # (step, p_b)
                for si, st in enumerate(steps):
                    g = st[0]
                    if g not in yt2_of:
                        # drain previous group before its bank is reused
                        if pend is not None and pend[0][0] != g:
                            emit_pv(pend[0], pend[1], yt2_of[pend[0][0]])
                            pend = None
                        if g - 1 in yt2_of:
                            drain(g - 1, yt2_of[g - 1])
                        yt2_of[g] = psY.tile([128, 390], f32, tag="yt2",
                                             name="yt2")
                    p_b = emit_s(st)
                    if pend is not None:
                        emit_pv(pend[0], pend[1], yt2_of[pend[0][0]])
                    pend = (st, p_b)
                    if fillers and si % stride == stride - 1:
                        fillers.pop(0)()
                if pend is not None:
                    emit_pv(pend[0], pend[1], yt2_of[pend[0][0]])
                drain(HKV - 1, yt2_of[HKV - 1])
                for f in fillers:
                    f()

            _ysb_cache = {}

            def _ysb_for(s, qb):
                key = (s, qb)
                if key not in _ysb_cache:
                    _ysb_cache[key] = ysp.tile([128, 576], bf16, tag="ysb",
                                               name=f"ysb{s}_{qb}")
                return _ysb_cache[key]

            def finish_qb(s, qb):
                """transpose ysb -> ypr for one query-128-block of slot s."""
                ysb = _ysb_cache.pop((s, qb))
                # transposes borrow the psB (filler) bank at slot boundaries
                tpp = psB.tile([128, 640], bf16, tag="pjb", name="tpp")
                for p, (pc0, pl) in enumerate(MM):
                    nc.tensor.transpose(tpp[:pl, 128 * p:128 * (p + 1)],
                                        ysb[:, pc0:pc0 + pl],
                                        ident_t[:, :])
                for p, (pc0, pl) in enumerate(MM):
                    nc.vector.tensor_copy(
                        ypr[p][:pl, QB * s + 128 * qb:QB * s + 128 * qb + 128],
                        tpp[:pl, 128 * p:128 * (p + 1)])

            def attn_finish(s):
                for qb in range(2):
                    finish_qb(s, qb)

            def attn_slot(s, fillers=()):
                attn_groups(s, SLOT_GROUPS[s], fillers=fillers)
                attn_finish(s)

            # ---------------- output projection ------------------------
            def oproj_qi(qi, act_copy=False):
                    psr = psA.tile([128, 512], f32, tag="pja", name="pja")
                    for p, (pc0, pl) in enumerate(MM):
                        nc.tensor.matmul(psr[:, :],
                                         ypr[p][:pl, 128 * qi:128 * (qi + 1)],
                                         wo_r[p][:pl, 0:512],
                                         start=(p == 0), stop=(p == 4))
                    psr2 = psB.tile([128, 512], f32, tag="pjb", name="pjb")
                    for p, (pc0, pl) in enumerate(MM):
                        nc.tensor.matmul(psr2[:, :64],
                                         ypr[p][:pl, 128 * qi:128 * (qi + 1)],
                                         wo_r[p][:pl, 512:576],
                                         start=(p == 0), stop=(p == 4))
                    ost = ysp.tile([128, 576], f32, tag="ost", name="ost")
                    if act_copy:
                        nc.scalar.activation(ost[:, 0:512], psr[:], AF.Copy)
                    else:
                        nc.vector.tensor_copy(ost[:, 0:512], psr[:])
                    nc.vector.tensor_copy(ost[:, 512:576], psr2[:, :64])
                    dma_eng = nc.scalar if qi % 2 else nc.sync
                    dma_eng.dma_start(yT[128 * qi:128 * (qi + 1), :], ost[:])

            # ---------------- schedule ---------------------------------
            # KV windows run in order 1,2,0,3 so slots 3 and 2 (which need
            # chunks 4-11) unlock after two windows and the exp stream on
            # ACT starts early. Later projection windows and O-proj blocks
            # ride as fillers inside the exp-bound attention sections, and
            # slots 0 and 1 are split in two passes (SBUF spill) so their
            # exp work spreads across the whole kernel instead of the tail.
            attn = "attn" not in ablate
            opj = "oproj" not in ablate and attn
            for u in qproj_units(1, xw1):        # queries 512:1024
                u()
            for u in kvproj_units(1, xw1, act_copy=True):   # key chunks 4-7
                u()
            for u in kvproj_units(2, xw2, act_copy=True):   # key chunks 8-11
                u()
            if attn:
                attn_slot(3, fillers=qproj_units(0, xw0))   # chunks 6-9
                attn_slot(2, fillers=kvproj_units(0, xw0))  # chunks 4-11
                # slot 1 pass A: first two groups (chunks 4-11)
                attn_groups(1, SLOT_GROUPS[1][:2], spill=True, last=False,
                            fillers=[lambda: oproj_qi(6), lambda: oproj_qi(7)]
                            if opj else ())
            if attn:
                # slot 0 pass A: first two groups (chunks 2-9)
                attn_groups(0, SLOT_GROUPS[0][:2], spill=True, last=False,
                            fillers=kvproj_units(3, xw3))   # chunks 12-15
                # slot 1 pass B: last group (chunks 12,13 + diag 2,3)
                attn_groups(1, SLOT_GROUPS[1][2:], spill=True, first=False,
                            fillers=[lambda: oproj_qi(4), lambda: oproj_qi(5)]
                            if opj else ())
                attn_finish(1)
                # slot 0 pass B: last two groups (chunks 10-15, 0-1)
                attn_groups(0, SLOT_GROUPS[0][2:], spill=True, first=False,
                            fillers=[lambda: oproj_qi(2), lambda: oproj_qi(3)]
                            if opj else ())
                finish_qb(0, 0)
                if "oproj" not in ablate:
                    oproj_qi(0, act_copy=True)
                finish_qb(0, 1)
                if "oproj" not in ablate:
                    oproj_qi(1, act_copy=True)

    nc.compile()
    return nc


def _get_program():
    global _PROG
    if _PROG is None:
        _PROG = _build_program()
    return _PROG


def _neox_perm(nheads, swap=False):
    p = []
    for h in range(nheads):
        ev = [64 * h + 2 * j for j in range(32)]
        od = [64 * h + 2 * j + 1 for j in range(32)]
        p += (od + ev) if swap else (ev + od)
    return np.array(p)


_CONSTS = None


def _static_consts():
    """Input-independent per-core constants (tables, masks, key orders)."""
    global _CONSTS
    if _CONSTS is not None:
        return _CONSTS
    invf = THETA ** (-np.arange(32, dtype=np.float64) / 32)

    def tables(pos):
        ang = pos[None, :] * invf[:, None]
        cos, sin = np.cos(ang), np.sin(ang)
        c2 = np.tile(cos, (4, 1)).astype(np.float32)
        s2 = np.tile(np.vstack([-sin, sin]), (2, 1)).astype(np.float32)
        return c2, s2

    per_j = []
    for j in range(2):
        keypos = np.concatenate(
            [np.arange(QB * q, QB * (q + 1)) for q in KEYORDER[j]])
        qsel = keypos[:TQ]          # queries = first 1024 permuted keys
        c2k, s2k = tables(keypos.astype(np.float64))
        masks = np.zeros((N_MASK * 128, QB), np.float32)
        for s in range(4):
            grp = SLOT_GROUPS[s][-1]
            qpos = keypos[QB * s:QB * (s + 1)]
            for k, c in enumerate(grp):
                kpos = keypos[128 * c:128 * (c + 1)]
                mi = MASK_OFF[s] + k
                masks[mi * 128:(mi + 1) * 128] = (
                    kpos[:, None] <= qpos[None, :]).astype(np.float32)
        per_j.append((keypos, qsel,
                      c2k.astype(ml_dtypes.bfloat16),
                      s2k.astype(ml_dtypes.bfloat16),
                      masks.astype(ml_dtypes.bfloat16)))
    _CONSTS = per_j
    return _CONSTS


def _perm_matrix():
    """128x128 block-diag pair-swap (per 64 rows: swap 32-halves)."""
    p = np.zeros((128, 128), np.float32)
    for b in range(2):
        for i in range(32):
            p[64 * b + 32 + i, 64 * b + i] = 1.0        # out[m]=in[swap(m)]
            p[64 * b + i, 64 * b + 32 + i] = 1.0
    return p.astype(ml_dtypes.bfloat16)


def _host_prep(x, Wq, Wk, Wv, Wo):
    wqT = _rne12(Wq[_neox_perm(H)].T)
    wkT = _rne12(Wk[_neox_perm(HKV)].T)
    woT = Wo.T.astype(ml_dtypes.bfloat16)
    wvT = np.zeros((577, 260), np.float32)
    for g in range(HKV):
        wvT[:C, 65 * g:65 * g + 64] = Wv[64 * g:64 * g + 64].T
        wvT[576, 65 * g + 64] = 1.0
    wvT = _rne12(wvT)
    perm = _perm_matrix()
    ident = np.eye(128, dtype=np.float32).astype(ml_dtypes.bfloat16)

    per_j = _static_consts()
    x = _rne12(x)
    ones = np.ones((1, T), np.float32)
    in_maps = []
    core_meta = []
    for b in range(B):
        xbT = x[b].T
        for j in range(2):
            keypos, qsel, c2k, s2k, masks = per_j[j]
            xkT = np.vstack([xbT[:, keypos], ones])
            in_maps.append({
                "xkT": xkT,
                "wqT": wqT, "wkT": wkT, "wvT": wvT, "woT": woT,
                "c2k": c2k, "s2k": s2k,
                "masks": masks, "perm": perm, "ident": ident,
            })
            core_meta.append((b, qsel))
    return in_maps, core_meta


def kernel(x, Wq, Wk, Wv, Wo):
    x = np.asarray(x, np.float32)
    Wq = np.asarray(Wq, np.float32)
    Wk = np.asarray(Wk, np.float32)
    Wv = np.asarray(Wv, np.float32)
    Wo = np.asarray(Wo, np.float32)

    from concourse.bass_utils import run_bass_kernel_spmd

    nc = _get_program()
    in_maps, core_meta = _host_prep(x, Wq, Wk, Wv, Wo)
    res = run_bass_kernel_spmd(nc, in_maps, list(range(8)))

    out = np.empty((B, T, C), np.float32)
    for core, (b, qsel) in enumerate(core_meta):
        out[b, qsel, :] = res.results[core]["yT"]
    return out
